# revision 30
# baseline (speedup 1.0000x reference)
"""Trainium2 Bass kernel for GQA attention (prefill), SPMD over 8 NeuronCores.

Sharding: tensor-parallel over heads (4-way) x data-parallel over batch (2-way).
Core c handles batch c//4 and head-group c%4 (8 q-heads / 2 kv-heads of the
32/8 global heads). Each core computes a full [S, D] partial of the output
projection (wo row-parallel); the 4 partials per batch are summed on host
during unsharding.

v2 layout notes (vs the v1 baseline):
- All DRAM inputs are host-packed so each logical tensor loads with ONE
  DMA (weights) or one DMA per q-chunk (x): the shared HWDGE descriptor
  unit serializes DMA issue at ~625ns each, so 190 DMAs/iter was ~119us
  of hidden serialization. v2 issues ~30.
- Scores for the two heads of a jt-pair go into one 2-bank PSUM tile
  [128, 2, 512]; softmax exp is ONE activation op per (hp, kb) instead of
  two+, halving the ~150ns/op fixed overhead count on the ACT engine.
- PSUM is budgeted exactly: 2x stp pairs (4 banks) + 1 pv pair (2 banks)
  + 1 shared proj/outproj/transpose slot (2 banks) = 8 banks.
- The output projection of chunk c-1 and the QKV projection of chunk c+1
  are emitted interleaved with attention of chunk c so the PE always has
  independent work while ACT catches up on exps / DVE normalizes.
- Output is written bf16 (host upcasts + sums partials in f32).

The [S, S] additive mask is handled by classifying each 128x128 block on
host (SKIP / ZERO / GENERAL as in v1); GENERAL blocks ship a transposed,
pre-scaled copy duplicated x2 so one DVE add covers both heads of a pair.
"""

import numpy as np
import ml_dtypes

import concourse.bacc as bacc
import concourse.mybir as mybir
import concourse.tile as tile
from concourse.bass_utils import run_bass_kernel_spmd

# Problem shape (hardcoded per contract).
B, S, D = 2, 2048, 2048
N_HEADS, N_KV_HEADS, HEAD_DIM = 32, 8, 64
TP = 4            # head-group shards
N_CORES = 8
BLK = 128         # block size (partitions)
NB = S // BLK     # 16 blocks along seq
CHUNK = 512       # q-chunk (moving operand width)
NCH = S // CHUNK  # 4 q-chunks
H_LOC = N_HEADS // TP        # 8 q heads per core
KV_LOC = N_KV_HEADS // TP    # 2 kv heads per core
KVD = KV_LOC * HEAD_DIM      # 128
JD = H_LOC * HEAD_DIM        # 512 local head dims
NJT = JD // BLK              # 4 jt tiles
SCALE = 1.0 / float(np.sqrt(HEAD_DIM))

F32 = mybir.dt.float32
BF16 = mybir.dt.bfloat16

COMPUTE = "bf16"

# mask block classes
SKIP, ZERO, GENERAL = 0, 1, 2


def classify_mask(mask: np.ndarray):
    """Classify each [BLK, BLK] block; return (cls, idx, unique_blocks).

    unique_blocks[i] holds a transposed mask block pre-scaled by sqrt(hd) so
    the on-device exp((raw_scores + m') * 1/sqrt(hd)) equals the reference
    exp(raw_scores/sqrt(hd) + m).
    """
    cls = np.empty((NB, NB), dtype=np.int64)
    idx = np.full((NB, NB), -1, dtype=np.int64)
    uniq = []
    seen = {}
    for qi in range(NB):
        for kb in range(NB):
            blkm = mask[qi * BLK:(qi + 1) * BLK, kb * BLK:(kb + 1) * BLK]
            if np.all(blkm <= -1e8):
                cls[qi, kb] = SKIP
            elif not np.any(blkm):
                cls[qi, kb] = ZERO
            else:
                cls[qi, kb] = GENERAL
                key = blkm.tobytes()
                if key not in seen:
                    seen[key] = len(uniq)
                    uniq.append(np.ascontiguousarray(blkm.T) / SCALE)
                idx[qi, kb] = seen[key]
    if not uniq:
        uniq.append(np.zeros((BLK, BLK), dtype=np.float32))
    ublk = np.stack(uniq).astype(np.float32)
    return cls, idx, ublk


def build_program(cls, idx, n_ublk, iters=1, compute=COMPUTE, phases=("proj", "attn", "out")):
    DT = BF16 if compute == "bf16" else F32
    nc = bacc.Bacc("TRN2", target_bir_lowering=False, debug=False,
                   num_devices=N_CORES)

    xP = nc.dram_tensor("xP", [BLK, NCH, NB, CHUNK], DT, kind="ExternalInput").ap()
    wqP = nc.dram_tensor("wqP", [BLK, NB, JD], DT, kind="ExternalInput").ap()
    wkP = nc.dram_tensor("wkP", [BLK, NB, KVD], DT, kind="ExternalInput").ap()
    wvP = nc.dram_tensor("wvP", [BLK, NB, KVD], DT, kind="ExternalInput").ap()
    woP = nc.dram_tensor("woP", [BLK, NJT, D], DT, kind="ExternalInput").ap()
    identP = nc.dram_tensor("identP", [BLK, HEAD_DIM], DT, kind="ExternalInput").ap()
    maskP = nc.dram_tensor("maskP", [BLK, n_ublk, 2, BLK], DT, kind="ExternalInput").ap()
    identFP = nc.dram_tensor("identFP", [BLK, BLK], DT, kind="ExternalInput").ap()
    outD = nc.dram_tensor("out", [S, D], BF16, kind="ExternalOutput").ap()

    with tile.TileContext(nc) as tc:
        with (
            tc.tile_pool(name="wp", bufs=1) as wp,       # resident weights/consts
            tc.tile_pool(name="kvp", bufs=1) as kvp,     # resident KT/V
            tc.tile_pool(name="xp", bufs=2) as xp,       # streaming x chunks
            tc.tile_pool(name="qp", bufs=2) as qp,       # QT pairs
            tc.tile_pool(name="pp", bufs=6) as pp,       # P pairs
            tc.tile_pool(name="mp", bufs=2) as mp,       # misc small
            tc.tile_pool(name="op", bufs=2) as op,       # out staging
            tc.tile_pool(name="psS", bufs=2, space="PSUM") as psS,   # stp pairs
            tc.tile_pool(name="psJ", bufs=1, space="PSUM") as psJ,   # proj/outproj/vps
            tc.tile_pool(name="psV", bufs=1, space="PSUM") as psV,   # pv pairs / kv proj
        ):
            def body():
                # ---- resident tiles ----
                wq_all = wp.tile([BLK, NB, JD], DT, tag="wq", name="wq_all")
                wk_all = wp.tile([BLK, NB, KVD], DT, tag="wk", name="wk_all")
                wv_all = wp.tile([BLK, NB, KVD], DT, tag="wv", name="wv_all")
                wo_all = wp.tile([BLK, NJT, D], DT, tag="wo", name="wo_all")
                ident = wp.tile([BLK, HEAD_DIM], DT, tag="ident", name="ident")
                identF = wp.tile([BLK, BLK], DT, tag="identF", name="identF")
                mk_all = wp.tile([BLK, n_ublk, 2, BLK], DT, tag="mk", name="mk_all")
                # wq in halves so the first Q pass can start sooner; wo last
                # (first needed at chunk 1's outproj interleave). x chunk 0 is
                # emitted before these (see below) so it wins HWDGE arbitration.
                def emit_w_dmas():
                    nc.scalar.dma_start(wq_all[:, :, 0:JD // 2],
                                        wqP[:, :, 0:JD // 2])
                    nc.scalar.dma_start(wq_all[:, :, JD // 2:JD],
                                        wqP[:, :, JD // 2:JD])
                    nc.scalar.dma_start(wk_all[:, :, :], wkP)
                    nc.scalar.dma_start(wv_all[:, :, :], wvP)
                    nc.scalar.dma_start(ident[:, :], identP)
                    nc.scalar.dma_start(identF[:, :], identFP)
                    nc.scalar.dma_start(mk_all[:, :, :, :], maskP)
                    nc.scalar.dma_start(wo_all[:, :, :], woP)

                # KT_all rows 0:64 hold K^T; rows 64:128 are a duplicate so
                # the sub=1 matmul's lhsT/rhs partition bases can match.
                KT_all = kvp.tile([BLK, KV_LOC, S], DT, tag="kt", name="KT_all")
                V_sb = [[kvp.tile([BLK, HEAD_DIM + 1], DT, tag=f"v{kv}_{kb}",
                                  name=f"v{kv}_{kb}")
                         for kb in range(NB)] for kv in range(KV_LOC)]

                xt_t = [None] * NCH
                qt_next = [None, None]   # jt-pair tiles, ping-pong via bufs=2

                def emit_x_dma(c, split=False):
                    xt = xp.tile([BLK, NB, CHUNK], DT, tag="xt", name="xt")
                    if split:
                        # halves: the first Q-pass matmul (db 0) unblocks after
                        # the first half lands.
                        nc.sync.dma_start(xt[:, 0:NB // 2, :], xP[:, c, 0:NB // 2, :])
                        nc.sync.dma_start(xt[:, NB // 2:NB, :], xP[:, c, NB // 2:NB, :])
                    else:
                        nc.sync.dma_start(xt[:, :, :], xP[:, c, :, :])
                    xt_t[c] = xt

                def emit_projQ(c, jp):
                    qt_ps = psJ.tile([BLK, 2, CHUNK], F32, tag="pj", name="qt_ps")
                    xt = xt_t[c]
                    for db in range(NB):
                        st_, sp_ = (db == 0), (db == NB - 1)
                        for jh in range(2):
                            jt = jp * 2 + jh
                            nc.tensor.matmul(
                                qt_ps[:, jh, :],
                                wq_all[:, db, jt * BLK:(jt + 1) * BLK],
                                xt[:, db, :], start=st_, stop=sp_)
                    qsb = qp.tile([BLK, 2, CHUNK], DT, tag=f"qt{jp}", name=f"qt{jp}")
                    nc.vector.tensor_copy(qsb[:, :, :], qt_ps[:, :, :])
                    qt_next[jp] = qsb

                def emit_projKV(c):
                    kv_ps = psV.tile([BLK, 2, CHUNK], F32, tag="pv", name="kv_ps")
                    xt = xt_t[c]
                    for db in range(NB):
                        st_, sp_ = (db == 0), (db == NB - 1)
                        nc.tensor.matmul(kv_ps[:, 0, :], wk_all[:, db, :],
                                         xt[:, db, :], start=st_, stop=sp_)
                        nc.tensor.matmul(kv_ps[:, 1, :], wv_all[:, db, :],
                                         xt[:, db, :], start=st_, stop=sp_)
                    sl = slice(c * CHUNK, (c + 1) * CHUNK)
                    for kv in range(KV_LOC):
                        nc.vector.tensor_copy(
                            KT_all[0:HEAD_DIM, kv, sl],
                            kv_ps[kv * HEAD_DIM:(kv + 1) * HEAD_DIM, 0, :])
                    nc.sync.dma_start(KT_all[HEAD_DIM:2 * HEAD_DIM, :, sl],
                                      KT_all[0:HEAD_DIM, :, sl])
                    vt_stage = mp.tile([BLK, CHUNK], DT, tag="vt", name="vt_stage")
                    nc.vector.tensor_copy(vt_stage[:, :], kv_ps[:, 1, :])
                    for kv in range(KV_LOC):
                        r0 = kv * HEAD_DIM
                        for kk in range(CHUNK // BLK):
                            kb = c * (CHUNK // BLK) + kk
                            v_ps = psJ.tile([BLK, HEAD_DIM], DT, tag="pj", name="v_ps")
                            nc.tensor.transpose(
                                v_ps[:, :],
                                vt_stage[r0:r0 + HEAD_DIM, kk * BLK:(kk + 1) * BLK],
                                ident[r0:r0 + HEAD_DIM, 0:HEAD_DIM])
                            nc.vector.tensor_copy(V_sb[kv][kb][:, 0:HEAD_DIM],
                                                  v_ps[:, :])
                            nc.vector.memset(V_sb[kv][kb][:, HEAD_DIM:HEAD_DIM + 1],
                                             1.0)

                filler = []      # FIFO of deferred outproj emission steps

                def pop_filler(n=1):
                    for _ in range(n):
                        if filler:
                            filler.pop(0)()

                def emit_attn_hp(c, hp, qis, qt_cur, mult_engine):
                    kv = hp // 2
                    jp, jh = hp // 2, hp % 2
                    kbs = [kb for kb in range(NB)
                           if any(cls[qi, kb] != SKIP for qi in qis)]
                    pv_ps = psV.tile([BLK, 2, CHUNK], F32, tag="pv", name="pv_ps")

                    def emit_scores(n_kb, kb):
                        nsk = [bool(cls[qi, kb] != SKIP) for qi in qis]
                        first = nsk.index(True)
                        if n_kb > 0 and all(nsk[first:]):
                            off = first * BLK  # valid blocks are a suffix
                        else:
                            off = 0
                        stp = psS.tile([BLK, 2, CHUNK], F32, tag="sp", name="stp")
                        for sub in range(2):
                            jr = sub * HEAD_DIM
                            nc.tensor.matmul(
                                stp[:, sub, off:],
                                KT_all[jr:jr + HEAD_DIM, kv, kb * BLK:(kb + 1) * BLK],
                                qt_cur[jp][jr:jr + HEAD_DIM, jh, off:],
                                start=True, stop=True)
                        # mask add ON THE PE: accumulate I.T @ mask into the
                        # score psum — avoids a DVE hop in the scores->exp->PV
                        # chain (f32r identity matmul, 1 cycle/row).
                        for ql in range(off // BLK, len(qis)):
                            qi = qis[ql]
                            if cls[qi, kb] == GENERAL:
                                csl = slice(ql * BLK, (ql + 1) * BLK)
                                for sub in range(2):
                                    nc.tensor.matmul(
                                        stp[:, sub, csl], identF[:, :],
                                        mk_all[:, idx[qi, kb], sub, :],
                                        start=False, stop=True,
                                        skip_group_check=True)
                        p = pp.tile([BLK, 2, CHUNK], DT, tag="p", name="p")
                        nc.scalar.activation(
                            p[:, :, off:], stp[:, :, off:],
                            mybir.ActivationFunctionType.Exp, scale=SCALE)
                        for ql in range(off // BLK, len(qis)):
                            if cls[qis[ql], kb] == SKIP:
                                nc.vector.memset(
                                    p[:, :, ql * BLK:(ql + 1) * BLK], 0.0)
                        return p, off

                    def emit_pv(n_kb, kb, p, off):
                        for sub in range(2):
                            nc.tensor.matmul(
                                pv_ps[0:HEAD_DIM + 1, sub, off:],
                                V_sb[kv][kb][:, :], p[:, sub, off:],
                                start=(n_kb == 0), stop=(n_kb == len(kbs) - 1))

                    # Software-pipelined by one kb: PE never issues a PV that
                    # waits on the exp of the scores it just computed; filler
                    # steps (prev-chunk outproj) absorb the ACT-rate deficit.
                    pop_filler()
                    prev = None
                    for n_kb, kb in enumerate(kbs):
                        p, off = emit_scores(n_kb, kb)
                        if prev is not None:
                            emit_pv(*prev)
                            pop_filler()
                        prev = (n_kb, kb, p, off)
                    emit_pv(*prev)
                    # Copy PV out of PSUM immediately: frees the pv bank for the
                    # next hp-run ~4us earlier than running the whole normalize
                    # chain out of PSUM would.
                    pvs = mp.tile([HEAD_DIM + 1, 2, CHUNK], F32, tag="pvs",
                                  name="pvs")
                    nc.vector.tensor_copy(pvs[:, :, :], pv_ps[0:HEAD_DIM + 1, :, :])
                    recip = mp.tile([1, 2, CHUNK], F32, tag="recip", name="recip")
                    nc.vector.reciprocal(recip[:, :, :],
                                         pvs[HEAD_DIM:HEAD_DIM + 1, :, :])
                    bc = mp.tile([HEAD_DIM, 2, CHUNK], F32, tag="bc", name="bc")
                    nc.gpsimd.partition_broadcast(bc[:, :, :], recip[:, :, :])
                    aT = mp.tile([BLK, CHUNK], DT, tag=f"attnT{hp}",
                                 name=f"attnT{hp}")
                    for sub in range(2):
                        jr = sub * HEAD_DIM
                        mult_engine.tensor_tensor(
                            out=aT[jr:jr + HEAD_DIM, :],
                            in0=pvs[0:HEAD_DIM, sub, :], in1=bc[:, sub, :],
                            op=mybir.AluOpType.mult)
                    return aT

                def emit_outproj(qi, aTs, pools=None, defer=False):
                    ql = qi % (CHUNK // BLK)
                    state = {}

                    def st_alloc():
                        state["o"] = op.tile([BLK, 2, 2, CHUNK], BF16, tag="o",
                                             name="o_big")

                    def st_mm(eh, jt, pool, ptag):
                        if jt == 0:
                            state["acc"] = pool.tile([BLK, 2, CHUNK], F32,
                                                     tag=ptag, name="acc")
                        for ei in range(2):
                            et = eh * 2 + ei
                            nc.tensor.matmul(
                                state["acc"][:, ei, :],
                                aTs[jt][:, ql * BLK:(ql + 1) * BLK],
                                wo_all[:, jt, et * CHUNK:(et + 1) * CHUNK],
                                start=(jt == 0), stop=(jt == NJT - 1))

                    def st_copy(eh):
                        nc.vector.tensor_copy(state["o"][:, eh, :, :],
                                              state["acc"][:, :, :])

                    def st_dma():
                        nc.sync.dma_start(outD[qi * BLK:(qi + 1) * BLK, :],
                                          state["o"][:, :, :, :])

                    steps = [st_alloc]
                    for eh in range(2):
                        if pools is None:
                            pool, ptag = psJ, "pj"
                        else:
                            pool, ptag = pools[eh % len(pools)]
                        for jt in range(NJT):
                            steps.append(lambda eh=eh, jt=jt, pl=pool, pt=ptag:
                                         st_mm(eh, jt, pl, pt))
                        steps.append(lambda eh=eh: st_copy(eh))
                        if defer:
                            # spacers: give the copy time to free the psum slot
                            # before the next acc's first matmul issues on PE
                            steps.extend([lambda: None, lambda: None])
                    steps.append(st_dma)
                    if defer:
                        filler.extend(steps)
                    else:
                        for s in steps:
                            s()

                # ---- prologue: chunk 0 projections ----
                emit_x_dma(0, split=True)
                emit_w_dmas()
                if "proj" in phases:
                    emit_projQ(0, 0)
                    emit_projQ(0, 1)
                    emit_projKV(0)

                prev_aTs = None
                prev_qis = None
                for c in range(NCH):
                    qis = list(range(c * (CHUNK // BLK), (c + 1) * (CHUNK // BLK)))
                    qt_cur = list(qt_next)
                    if c + 1 < NCH:
                        emit_x_dma(c + 1)
                    mult_eng = nc.vector  # gpsimd tensor_tensor is ~7x slower on HW
                    aTs = []
                    for hp in range(H_LOC // 2):
                        if "attn" in phases:
                            if "out" in phases and prev_aTs is not None:
                                emit_outproj(prev_qis[hp], prev_aTs, defer=True)
                            aTs.append(emit_attn_hp(c, hp, qis, qt_cur, mult_eng))
                        if "proj" in phases and c + 1 < NCH:
                            if hp == 1:
                                emit_projQ(c + 1, 0)
                            elif hp == 2:
                                emit_projQ(c + 1, 1)
                            elif hp == 3:
                                emit_projKV(c + 1)
                    pop_filler(len(filler))   # drain before attnT ring reuse
                    prev_aTs, prev_qis = aTs, qis

                # epilogue: last chunk's outproj — attention psum is free, so
                # round-robin accs across all tags to pipeline the copies.
                if "attn" in phases and "out" in phases and prev_aTs is not None:
                    rr = [[(psS, "sp"), (psV, "pv")], [(psJ, "pj"), (psS, "sp")]]
                    for i, qi in enumerate(prev_qis):
                        emit_outproj(qi, prev_aTs, pools=rr[i % 2])

            if iters == 1:
                body()
            else:
                hints = (mybir.EngineType.PE, mybir.EngineType.DVE,
                         mybir.EngineType.Activation, mybir.EngineType.SP,
                         mybir.EngineType.Pool)
                with tc.For_i(0, iters, hint_engines=hints):
                    body()
    nc.compile()
    return nc


def make_in_maps(x, wq, wk, wv, wo, ublk, compute=COMPUTE):
    npdt = ml_dtypes.bfloat16 if compute == "bf16" else np.float32
    ident = np.tile(np.eye(HEAD_DIM, dtype=np.float32), (2, 1)).astype(npdt)
    identf = np.eye(BLK, dtype=np.float32).astype(npdt)
    n_ublk = len(ublk)
    # maskP [128, n_ublk, 2, 128]: each transposed+prescaled block duplicated
    mk = np.repeat(ublk[:, None, :, :], 2, axis=1)          # [n, 2, 128, 128]
    mk = np.ascontiguousarray(mk.transpose(2, 0, 1, 3))     # [128, n, 2, 128]
    in_maps = []
    for cc in range(N_CORES):
        b, g = cc // TP, cc % TP
        xb = x[b]                                           # [S, D]
        # xP[p, c, db, j] = x[c*CHUNK+j, db*BLK+p]
        xp = xb.reshape(NCH, CHUNK, NB, BLK).transpose(3, 0, 2, 1)
        wql = wq[g * JD:(g + 1) * JD, :]                    # [JD, D]
        # wqP[p, db, jd] = wq[g*JD+jd, db*BLK+p]
        wqp = wql.reshape(JD, NB, BLK).transpose(2, 1, 0)
        wkl = wk[g * KVD:(g + 1) * KVD, :]
        wkp = wkl.reshape(KVD, NB, BLK).transpose(2, 1, 0)
        wvl = wv[g * KVD:(g + 1) * KVD, :]
        wvp = wvl.reshape(KVD, NB, BLK).transpose(2, 1, 0)
        wol = wo[:, g * JD:(g + 1) * JD]                    # [D, JD]
        # woP[p, jt, e] = wo[e, g*JD + jt*BLK + p]
        wop = wol.reshape(D, NJT, BLK).transpose(2, 1, 0)
        in_maps.append({
            "xP": np.ascontiguousarray(xp).astype(npdt),
            "wqP": np.ascontiguousarray(wqp).astype(npdt),
            "wkP": np.ascontiguousarray(wkp).astype(npdt),
            "wvP": np.ascontiguousarray(wvp).astype(npdt),
            "woP": np.ascontiguousarray(wop).astype(npdt),
            "identP": ident,
            "identFP": identf,
            "maskP": mk.astype(npdt),
        })
    return in_maps


def kernel(x, wq, wk, wv, wo, mask, start_pos):
    x = np.asarray(x, dtype=np.float32)
    wq = np.asarray(wq, dtype=np.float32)
    wk = np.asarray(wk, dtype=np.float32)
    wv = np.asarray(wv, dtype=np.float32)
    wo = np.asarray(wo, dtype=np.float32)
    mask = np.asarray(mask, dtype=np.float32)

    cls, idx, ublk = classify_mask(mask)
    nc = build_program(cls, idx, len(ublk), iters=1)
    in_maps = make_in_maps(x, wq, wk, wv, wo, ublk)
    res = run_bass_kernel_spmd(nc, in_maps, core_ids=list(range(N_CORES)),
                               trace=False)
    out = np.zeros((B, S, D), dtype=np.float32)
    for c in range(N_CORES):
        out[c // TP] += res.results[c]["out"].astype(np.float32)
    return out


# revision 38
# speedup vs baseline: 1.0275x; 1.0275x over previous
"""Trainium2 Bass kernel for GQA attention (prefill), SPMD over 8 NeuronCores.

Sharding: tensor-parallel over heads (4-way) x data-parallel over batch (2-way).
Core c handles batch c//4 and head-group c%4 (8 q-heads / 2 kv-heads of the
32/8 global heads). Each core computes a full [S, D] partial of the output
projection (wo row-parallel); the 4 partials per batch are summed on host
during unsharding.

v2 layout notes (vs the v1 baseline):
- All DRAM inputs are host-packed so each logical tensor loads with ONE
  DMA (weights) or one DMA per q-chunk (x): the shared HWDGE descriptor
  unit serializes DMA issue at ~625ns each, so 190 DMAs/iter was ~119us
  of hidden serialization. v2 issues ~30.
- Scores for the two heads of a jt-pair go into one 2-bank PSUM tile
  [128, 2, 512]; softmax exp is ONE activation op per (hp, kb) instead of
  two+, halving the ~150ns/op fixed overhead count on the ACT engine.
- PSUM is budgeted exactly: 2x stp pairs (4 banks) + 1 pv pair (2 banks)
  + 1 shared proj/outproj/transpose slot (2 banks) = 8 banks.
- The output projection of chunk c-1 and the QKV projection of chunk c+1
  are emitted interleaved with attention of chunk c so the PE always has
  independent work while ACT catches up on exps / DVE normalizes.
- Output is written bf16 (host upcasts + sums partials in f32).

The [S, S] additive mask is handled by classifying each 128x128 block on
host (SKIP / ZERO / GENERAL as in v1); GENERAL blocks ship a transposed,
pre-scaled copy duplicated x2 so one DVE add covers both heads of a pair.
"""

import numpy as np
import ml_dtypes

import concourse.bacc as bacc
import concourse.mybir as mybir
import concourse.tile as tile
from concourse.bass_utils import run_bass_kernel_spmd

# Problem shape (hardcoded per contract).
B, S, D = 2, 2048, 2048
N_HEADS, N_KV_HEADS, HEAD_DIM = 32, 8, 64
TP = 4            # head-group shards
N_CORES = 8
BLK = 128         # block size (partitions)
NB = S // BLK     # 16 blocks along seq
CHUNK = 512       # q-chunk (moving operand width)
NCH = S // CHUNK  # 4 q-chunks
H_LOC = N_HEADS // TP        # 8 q heads per core
KV_LOC = N_KV_HEADS // TP    # 2 kv heads per core
KVD = KV_LOC * HEAD_DIM      # 128
JD = H_LOC * HEAD_DIM        # 512 local head dims
NJT = JD // BLK              # 4 jt tiles
SCALE = 1.0 / float(np.sqrt(HEAD_DIM))

F32 = mybir.dt.float32
BF16 = mybir.dt.bfloat16

COMPUTE = "bf16"

# mask block classes
SKIP, ZERO, GENERAL = 0, 1, 2

# tuning knobs (see compare.py); safe defaults
OPTS = {}


def classify_mask(mask: np.ndarray):
    """Classify each [BLK, BLK] block; return (cls, idx, unique_blocks).

    unique_blocks[i] holds a transposed mask block pre-scaled by sqrt(hd) so
    the on-device exp((raw_scores + m') * 1/sqrt(hd)) equals the reference
    exp(raw_scores/sqrt(hd) + m).
    """
    cls = np.empty((NB, NB), dtype=np.int64)
    idx = np.full((NB, NB), -1, dtype=np.int64)
    uniq = []
    seen = {}
    for qi in range(NB):
        for kb in range(NB):
            blkm = mask[qi * BLK:(qi + 1) * BLK, kb * BLK:(kb + 1) * BLK]
            if np.all(blkm <= -1e8):
                cls[qi, kb] = SKIP
            elif not np.any(blkm):
                cls[qi, kb] = ZERO
            else:
                cls[qi, kb] = GENERAL
                key = blkm.tobytes()
                if key not in seen:
                    seen[key] = len(uniq)
                    uniq.append(np.ascontiguousarray(blkm.T) / SCALE)
                idx[qi, kb] = seen[key]
    if not uniq:
        uniq.append(np.zeros((BLK, BLK), dtype=np.float32))
    ublk = np.stack(uniq).astype(np.float32)
    return cls, idx, ublk


def build_program(cls, idx, n_ublk, iters=1, compute=COMPUTE, phases=("proj", "attn", "out")):
    DT = BF16 if compute == "bf16" else F32
    nc = bacc.Bacc("TRN2", target_bir_lowering=False, debug=False,
                   num_devices=N_CORES)

    xP = nc.dram_tensor("xP", [BLK, NCH, NB, CHUNK], DT, kind="ExternalInput").ap()
    wqP = nc.dram_tensor("wqP", [BLK, NB, JD], DT, kind="ExternalInput").ap()
    wkP = nc.dram_tensor("wkP", [BLK, NB, KVD], DT, kind="ExternalInput").ap()
    wvP = nc.dram_tensor("wvP", [BLK, NB, KVD], DT, kind="ExternalInput").ap()
    woP = nc.dram_tensor("woP", [BLK, NJT, D], DT, kind="ExternalInput").ap()
    identP = nc.dram_tensor("identP", [BLK, HEAD_DIM], DT, kind="ExternalInput").ap()
    maskP = nc.dram_tensor("maskP", [BLK, n_ublk, 2, BLK], DT, kind="ExternalInput").ap()
    identFP = nc.dram_tensor("identFP", [BLK, BLK], DT, kind="ExternalInput").ap()
    outD = nc.dram_tensor("out", [S, D], BF16, kind="ExternalOutput").ap()

    with tile.TileContext(nc) as tc:
        with (
            tc.tile_pool(name="wp", bufs=1) as wp,       # resident weights/consts
            tc.tile_pool(name="kvp", bufs=1) as kvp,     # resident KT/V
            tc.tile_pool(name="xp", bufs=2) as xp,       # streaming x chunks
            tc.tile_pool(name="qp", bufs=2) as qp,       # QT pairs
            tc.tile_pool(name="pp", bufs=int(OPTS.get("ppbufs", 6))) as pp,  # P pairs
            tc.tile_pool(name="mp", bufs=2) as mp,       # misc small
            tc.tile_pool(name="op", bufs=2) as op,       # out staging
            tc.tile_pool(name="psS", bufs=2, space="PSUM") as psS,   # stp pairs
            tc.tile_pool(name="psJ", bufs=1, space="PSUM") as psJ,   # proj/outproj/vps
            tc.tile_pool(name="psV", bufs=1, space="PSUM") as psV,   # pv pairs / kv proj
        ):
            def body():
                # ---- resident tiles ----
                wq_all = wp.tile([BLK, NB, JD], DT, tag="wq", name="wq_all")
                wk_all = wp.tile([BLK, NB, KVD], DT, tag="wk", name="wk_all")
                wv_all = wp.tile([BLK, NB, KVD], DT, tag="wv", name="wv_all")
                wo_all = wp.tile([BLK, NJT, D], DT, tag="wo", name="wo_all")
                ident = wp.tile([BLK, HEAD_DIM], DT, tag="ident", name="ident")
                identF = wp.tile([BLK, BLK], DT, tag="identF", name="identF")
                mk_all = wp.tile([BLK, n_ublk, 2, BLK], DT, tag="mk", name="mk_all")
                # wq in halves so the first Q pass can start sooner; wo last
                # (first needed at chunk 1's outproj interleave). x chunk 0 is
                # emitted before these (see below) so it wins HWDGE arbitration.
                def emit_w_dmas():
                    nc.scalar.dma_start(wq_all[:, :, 0:JD // 2],
                                        wqP[:, :, 0:JD // 2])
                    nc.scalar.dma_start(wq_all[:, :, JD // 2:JD],
                                        wqP[:, :, JD // 2:JD])
                    nc.scalar.dma_start(wk_all[:, :, :], wkP)
                    nc.scalar.dma_start(wv_all[:, :, :], wvP)
                    nc.scalar.dma_start(ident[:, :], identP)
                    nc.scalar.dma_start(identF[:, :], identFP)
                    nc.scalar.dma_start(mk_all[:, :, :, :], maskP)
                    nc.scalar.dma_start(wo_all[:, :, :], woP)

                # KT_all rows 0:64 hold K^T; rows 64:128 are a duplicate so
                # the sub=1 matmul's lhsT/rhs partition bases can match.
                KT_all = kvp.tile([BLK, KV_LOC, S], DT, tag="kt", name="KT_all")
                V_sb = [[kvp.tile([BLK, HEAD_DIM + 1], DT, tag=f"v{kv}_{kb}",
                                  name=f"v{kv}_{kb}")
                         for kb in range(NB)] for kv in range(KV_LOC)]

                xt_t = [None] * NCH
                qt_next = [None, None]   # jt-pair tiles, ping-pong via bufs=2

                def emit_x_dma(c, split=False):
                    xt = xp.tile([BLK, NB, CHUNK], DT, tag="xt", name="xt")
                    if split:
                        # halves: the first Q-pass matmul (db 0) unblocks after
                        # the first half lands.
                        nc.sync.dma_start(xt[:, 0:NB // 2, :], xP[:, c, 0:NB // 2, :])
                        nc.sync.dma_start(xt[:, NB // 2:NB, :], xP[:, c, NB // 2:NB, :])
                    else:
                        nc.sync.dma_start(xt[:, :, :], xP[:, c, :, :])
                    xt_t[c] = xt

                def emit_projQ(c, jp):
                    qt_ps = psJ.tile([BLK, 2, CHUNK], F32, tag="pj", name="qt_ps")
                    xt = xt_t[c]
                    for db in range(NB):
                        st_, sp_ = (db == 0), (db == NB - 1)
                        for jh in range(2):
                            jt = jp * 2 + jh
                            nc.tensor.matmul(
                                qt_ps[:, jh, :],
                                wq_all[:, db, jt * BLK:(jt + 1) * BLK],
                                xt[:, db, :], start=st_, stop=sp_)
                    qsb = qp.tile([BLK, 2, CHUNK], DT, tag=f"qt{jp}", name=f"qt{jp}")
                    if OPTS.get("qtsplit"):
                        for jh in range(2):
                            nc.vector.tensor_copy(qsb[:, jh, :], qt_ps[:, jh, :])
                    else:
                        nc.vector.tensor_copy(qsb[:, :, :], qt_ps[:, :, :])
                    qt_next[jp] = qsb

                def emit_projKV(c):
                    kv_ps = psV.tile([BLK, 2, CHUNK], F32, tag="pv", name="kv_ps")
                    xt = xt_t[c]
                    for db in range(NB):
                        st_, sp_ = (db == 0), (db == NB - 1)
                        nc.tensor.matmul(kv_ps[:, 0, :], wk_all[:, db, :],
                                         xt[:, db, :], start=st_, stop=sp_)
                        nc.tensor.matmul(kv_ps[:, 1, :], wv_all[:, db, :],
                                         xt[:, db, :], start=st_, stop=sp_)
                    sl = slice(c * CHUNK, (c + 1) * CHUNK)
                    for kv in range(KV_LOC):
                        nc.vector.tensor_copy(
                            KT_all[0:HEAD_DIM, kv, sl],
                            kv_ps[kv * HEAD_DIM:(kv + 1) * HEAD_DIM, 0, :])
                    nc.sync.dma_start(KT_all[HEAD_DIM:2 * HEAD_DIM, :, sl],
                                      KT_all[0:HEAD_DIM, :, sl])
                    vt_stage = mp.tile([BLK, CHUNK], DT, tag="vt", name="vt_stage")
                    nc.vector.tensor_copy(vt_stage[:, :], kv_ps[:, 1, :])
                    for kv in range(KV_LOC):
                        r0 = kv * HEAD_DIM
                        for kk in range(CHUNK // BLK):
                            kb = c * (CHUNK // BLK) + kk
                            v_ps = psJ.tile([BLK, HEAD_DIM], DT, tag="pj", name="v_ps")
                            nc.tensor.transpose(
                                v_ps[:, :],
                                vt_stage[r0:r0 + HEAD_DIM, kk * BLK:(kk + 1) * BLK],
                                ident[r0:r0 + HEAD_DIM, 0:HEAD_DIM])
                            nc.vector.tensor_copy(V_sb[kv][kb][:, 0:HEAD_DIM],
                                                  v_ps[:, :])
                            nc.vector.memset(V_sb[kv][kb][:, HEAD_DIM:HEAD_DIM + 1],
                                             1.0)

                filler = []      # FIFO of deferred outproj emission steps

                def pop_filler(n=1):
                    for _ in range(n):
                        if filler:
                            filler.pop(0)()

                def emit_attn_hp(c, hp, qis, qt_cur, mult_engine):
                    kv = hp // 2
                    jp, jh = hp // 2, hp % 2
                    kbs = [kb for kb in range(NB)
                           if any(cls[qi, kb] != SKIP for qi in qis)]
                    pv_ps = psV.tile([BLK, 2, CHUNK], F32, tag="pv", name="pv_ps")

                    def emit_scores(n_kb, kb):
                        nsk = [bool(cls[qi, kb] != SKIP) for qi in qis]
                        first = nsk.index(True)
                        if n_kb > 0 and all(nsk[first:]):
                            off = first * BLK  # valid blocks are a suffix
                        else:
                            off = 0
                        stp = psS.tile([BLK, 2, CHUNK], F32, tag="sp", name="stp")
                        for sub in range(2):
                            jr = sub * HEAD_DIM
                            nc.tensor.matmul(
                                stp[:, sub, off:],
                                KT_all[jr:jr + HEAD_DIM, kv, kb * BLK:(kb + 1) * BLK],
                                qt_cur[jp][jr:jr + HEAD_DIM, jh, off:],
                                start=True, stop=True)
                        # mask add ON THE PE: accumulate I.T @ mask into the
                        # score psum — avoids a DVE hop in the scores->exp->PV
                        # chain (f32r identity matmul, 1 cycle/row).
                        for ql in range(off // BLK, len(qis)):
                            qi = qis[ql]
                            if cls[qi, kb] == GENERAL:
                                csl = slice(ql * BLK, (ql + 1) * BLK)
                                for sub in range(2):
                                    nc.tensor.matmul(
                                        stp[:, sub, csl], identF[:, :],
                                        mk_all[:, idx[qi, kb], sub, :],
                                        start=False, stop=True,
                                        skip_group_check=True)
                        p = pp.tile([BLK, 2, CHUNK], DT, tag="p", name="p")
                        nc.scalar.activation(
                            p[:, :, off:], stp[:, :, off:],
                            mybir.ActivationFunctionType.Exp, scale=SCALE)
                        for ql in range(off // BLK, len(qis)):
                            if cls[qis[ql], kb] == SKIP:
                                nc.vector.memset(
                                    p[:, :, ql * BLK:(ql + 1) * BLK], 0.0)
                        return p, off

                    def emit_pv(n_kb, kb, p, off):
                        for sub in range(2):
                            nc.tensor.matmul(
                                pv_ps[0:HEAD_DIM + 1, sub, off:],
                                V_sb[kv][kb][:, :], p[:, sub, off:],
                                start=(n_kb == 0), stop=(n_kb == len(kbs) - 1))

                    # Software-pipelined: PV lags scores by `pvlag` kbs (PV only
                    # needs the SBUF p tile, so lag is free in PSUM) — hides the
                    # ACT latency + semaphore hops; filler steps (prev-chunk
                    # outproj) absorb the ACT-rate deficit.
                    lag = int(OPTS.get("pvlag", 2))
                    pop_filler()
                    pending = []
                    for n_kb, kb in enumerate(kbs):
                        p, off = emit_scores(n_kb, kb)
                        pending.append((n_kb, kb, p, off))
                        if len(pending) > lag:
                            emit_pv(*pending.pop(0))
                            pop_filler()
                    for pr in pending:
                        emit_pv(*pr)
                    # Copy PV out of PSUM immediately: frees the pv bank for the
                    # next hp-run ~4us earlier than running the whole normalize
                    # chain out of PSUM would.
                    pvs = mp.tile([HEAD_DIM + 1, 2, CHUNK], F32, tag="pvs",
                                  name="pvs")
                    nc.vector.tensor_copy(pvs[:, :, :], pv_ps[0:HEAD_DIM + 1, :, :])
                    recip = mp.tile([1, 2, CHUNK], F32, tag="recip", name="recip")
                    nc.vector.reciprocal(recip[:, :, :],
                                         pvs[HEAD_DIM:HEAD_DIM + 1, :, :])
                    bc = mp.tile([HEAD_DIM, 2, CHUNK], F32, tag="bc", name="bc")
                    nc.gpsimd.partition_broadcast(bc[:, :, :], recip[:, :, :])
                    aT = mp.tile([BLK, CHUNK], DT, tag=f"attnT{hp}",
                                 name=f"attnT{hp}")
                    for sub in range(2):
                        jr = sub * HEAD_DIM
                        mult_engine.tensor_tensor(
                            out=aT[jr:jr + HEAD_DIM, :],
                            in0=pvs[0:HEAD_DIM, sub, :], in1=bc[:, sub, :],
                            op=mybir.AluOpType.mult)
                    return aT

                def emit_outproj(qi, aTs, pools=None, defer=False):
                    ql = qi % (CHUNK // BLK)
                    state = {}

                    def st_alloc():
                        state["o"] = op.tile([BLK, 2, 2, CHUNK], BF16, tag="o",
                                             name="o_big")

                    def st_mm(eh, jt, pool, ptag):
                        if jt == 0:
                            state["acc"] = pool.tile([BLK, 2, CHUNK], F32,
                                                     tag=ptag, name="acc")
                        for ei in range(2):
                            et = eh * 2 + ei
                            nc.tensor.matmul(
                                state["acc"][:, ei, :],
                                aTs[jt][:, ql * BLK:(ql + 1) * BLK],
                                wo_all[:, jt, et * CHUNK:(et + 1) * CHUNK],
                                start=(jt == 0), stop=(jt == NJT - 1))

                    def st_copy(eh):
                        if OPTS.get("ocopy") == "scalar":
                            nc.scalar.copy(state["o"][:, eh, :, :],
                                           state["acc"][:, :, :])
                        else:
                            nc.vector.tensor_copy(state["o"][:, eh, :, :],
                                                  state["acc"][:, :, :])

                    def st_dma():
                        nc.sync.dma_start(outD[qi * BLK:(qi + 1) * BLK, :],
                                          state["o"][:, :, :, :])

                    steps = [st_alloc]
                    for eh in range(2):
                        if pools is None:
                            pool, ptag = psJ, "pj"
                        else:
                            pool, ptag = pools[eh % len(pools)]
                        for jt in range(NJT):
                            steps.append(lambda eh=eh, jt=jt, pl=pool, pt=ptag:
                                         st_mm(eh, jt, pl, pt))
                        steps.append(lambda eh=eh: st_copy(eh))
                        if defer:
                            # spacers: give the copy time to free the psum slot
                            # before the next acc's first matmul issues on PE
                            steps.extend([lambda: None] *
                                         int(OPTS.get("spacers", 2)))
                    steps.append(st_dma)
                    if defer:
                        filler.extend(steps)
                    else:
                        for s in steps:
                            s()

                # ---- prologue: chunk 0 projections ----
                emit_x_dma(0, split=True)
                emit_w_dmas()
                if "proj" in phases:
                    emit_projQ(0, 0)
                    emit_projQ(0, 1)
                    emit_projKV(0)

                prev_aTs = None
                prev_qis = None
                for c in range(NCH):
                    qis = list(range(c * (CHUNK // BLK), (c + 1) * (CHUNK // BLK)))
                    qt_cur = list(qt_next)
                    if c + 1 < NCH:
                        emit_x_dma(c + 1)
                    # gpsimd tensor_tensor measured ~7x slower than modeled
                    mult_eng = (nc.gpsimd if OPTS.get("mult") == "gpsimd"
                                else nc.vector)
                    aTs = []
                    for hp in range(H_LOC // 2):
                        if "attn" in phases:
                            if "out" in phases and prev_aTs is not None:
                                emit_outproj(prev_qis[hp], prev_aTs, defer=True)
                            aTs.append(emit_attn_hp(c, hp, qis, qt_cur, mult_eng))
                        if "proj" in phases and c + 1 < NCH:
                            if hp == 1:
                                emit_projQ(c + 1, 0)
                            elif hp == 2:
                                emit_projQ(c + 1, 1)
                            elif hp == 3:
                                emit_projKV(c + 1)
                    pop_filler(len(filler))   # drain before attnT ring reuse
                    prev_aTs, prev_qis = aTs, qis

                # epilogue: last chunk's outproj — attention psum is free, so
                # round-robin accs across all tags to pipeline the copies.
                if "attn" in phases and "out" in phases and prev_aTs is not None:
                    rr = [[(psS, "sp"), (psV, "pv")], [(psJ, "pj"), (psS, "sp")]]
                    for i, qi in enumerate(prev_qis):
                        emit_outproj(qi, prev_aTs, pools=rr[i % 2])

            if iters == 1:
                body()
            else:
                hints = (mybir.EngineType.PE, mybir.EngineType.DVE,
                         mybir.EngineType.Activation, mybir.EngineType.SP,
                         mybir.EngineType.Pool)
                unroll = int(OPTS.get("unroll", 1))
                if unroll > 1 and iters % unroll == 0:
                    with tc.For_i(0, iters // unroll, hint_engines=hints):
                        for _ in range(unroll):
                            body()
                else:
                    with tc.For_i(0, iters, hint_engines=hints):
                        body()
    nc.compile()
    return nc


def make_in_maps(x, wq, wk, wv, wo, ublk, compute=COMPUTE):
    npdt = ml_dtypes.bfloat16 if compute == "bf16" else np.float32
    ident = np.tile(np.eye(HEAD_DIM, dtype=np.float32), (2, 1)).astype(npdt)
    identf = np.eye(BLK, dtype=np.float32).astype(npdt)
    n_ublk = len(ublk)
    # maskP [128, n_ublk, 2, 128]: each transposed+prescaled block duplicated
    mk = np.repeat(ublk[:, None, :, :], 2, axis=1)          # [n, 2, 128, 128]
    mk = np.ascontiguousarray(mk.transpose(2, 0, 1, 3))     # [128, n, 2, 128]
    in_maps = []
    for cc in range(N_CORES):
        b, g = cc // TP, cc % TP
        xb = x[b]                                           # [S, D]
        # xP[p, c, db, j] = x[c*CHUNK+j, db*BLK+p]
        xp = xb.reshape(NCH, CHUNK, NB, BLK).transpose(3, 0, 2, 1)
        wql = wq[g * JD:(g + 1) * JD, :]                    # [JD, D]
        # wqP[p, db, jd] = wq[g*JD+jd, db*BLK+p]
        wqp = wql.reshape(JD, NB, BLK).transpose(2, 1, 0)
        wkl = wk[g * KVD:(g + 1) * KVD, :]
        wkp = wkl.reshape(KVD, NB, BLK).transpose(2, 1, 0)
        wvl = wv[g * KVD:(g + 1) * KVD, :]
        wvp = wvl.reshape(KVD, NB, BLK).transpose(2, 1, 0)
        wol = wo[:, g * JD:(g + 1) * JD]                    # [D, JD]
        # woP[p, jt, e] = wo[e, g*JD + jt*BLK + p]
        wop = wol.reshape(D, NJT, BLK).transpose(2, 1, 0)
        in_maps.append({
            "xP": np.ascontiguousarray(xp).astype(npdt),
            "wqP": np.ascontiguousarray(wqp).astype(npdt),
            "wkP": np.ascontiguousarray(wkp).astype(npdt),
            "wvP": np.ascontiguousarray(wvp).astype(npdt),
            "woP": np.ascontiguousarray(wop).astype(npdt),
            "identP": ident,
            "identFP": identf,
            "maskP": mk.astype(npdt),
        })
    return in_maps


def kernel(x, wq, wk, wv, wo, mask, start_pos):
    x = np.asarray(x, dtype=np.float32)
    wq = np.asarray(wq, dtype=np.float32)
    wk = np.asarray(wk, dtype=np.float32)
    wv = np.asarray(wv, dtype=np.float32)
    wo = np.asarray(wo, dtype=np.float32)
    mask = np.asarray(mask, dtype=np.float32)

    cls, idx, ublk = classify_mask(mask)
    nc = build_program(cls, idx, len(ublk), iters=1)
    in_maps = make_in_maps(x, wq, wk, wv, wo, ublk)
    res = run_bass_kernel_spmd(nc, in_maps, core_ids=list(range(N_CORES)),
                               trace=False)
    out = np.zeros((B, S, D), dtype=np.float32)
    for c in range(N_CORES):
        out[c // TP] += res.results[c]["out"].astype(np.float32)
    return out


# revision 41
# speedup vs baseline: 1.0662x; 1.0376x over previous
"""Trainium2 Bass kernel for GQA attention (prefill), SPMD over 8 NeuronCores.

Sharding: tensor-parallel over heads (4-way) x data-parallel over batch (2-way).
Core c handles batch c//4 and head-group c%4 (8 q-heads / 2 kv-heads of the
32/8 global heads). Each core computes a full [S, D] partial of the output
projection (wo row-parallel); the 4 partials per batch are summed on host
during unsharding.

v2 layout notes (vs the v1 baseline):
- All DRAM inputs are host-packed so each logical tensor loads with ONE
  DMA (weights) or one DMA per q-chunk (x): the shared HWDGE descriptor
  unit serializes DMA issue at ~625ns each, so 190 DMAs/iter was ~119us
  of hidden serialization. v2 issues ~30.
- Scores for the two heads of a jt-pair go into one 2-bank PSUM tile
  [128, 2, 512]; softmax exp is ONE activation op per (hp, kb) instead of
  two+, halving the ~150ns/op fixed overhead count on the ACT engine.
- PSUM is budgeted exactly: 2x stp pairs (4 banks) + 1 pv pair (2 banks)
  + 1 shared proj/outproj/transpose slot (2 banks) = 8 banks.
- The output projection of chunk c-1 and the QKV projection of chunk c+1
  are emitted interleaved with attention of chunk c so the PE always has
  independent work while ACT catches up on exps / DVE normalizes.
- Output is written bf16 (host upcasts + sums partials in f32).

The [S, S] additive mask is handled by classifying each 128x128 block on
host (SKIP / ZERO / GENERAL as in v1); GENERAL blocks ship a transposed,
pre-scaled copy duplicated x2 so one DVE add covers both heads of a pair.
"""

import numpy as np
import ml_dtypes

import concourse.bacc as bacc
import concourse.mybir as mybir
import concourse.tile as tile
from concourse.bass_utils import run_bass_kernel_spmd

# Problem shape (hardcoded per contract).
B, S, D = 2, 2048, 2048
N_HEADS, N_KV_HEADS, HEAD_DIM = 32, 8, 64
TP = 4            # head-group shards
N_CORES = 8
BLK = 128         # block size (partitions)
NB = S // BLK     # 16 blocks along seq
CHUNK = 512       # q-chunk (moving operand width)
NCH = S // CHUNK  # 4 q-chunks
H_LOC = N_HEADS // TP        # 8 q heads per core
KV_LOC = N_KV_HEADS // TP    # 2 kv heads per core
KVD = KV_LOC * HEAD_DIM      # 128
JD = H_LOC * HEAD_DIM        # 512 local head dims
NJT = JD // BLK              # 4 jt tiles
SCALE = 1.0 / float(np.sqrt(HEAD_DIM))

F32 = mybir.dt.float32
BF16 = mybir.dt.bfloat16

COMPUTE = "bf16"

# mask block classes
SKIP, ZERO, GENERAL = 0, 1, 2

# tuning knobs (see compare.py); safe defaults
OPTS = {}


def classify_mask(mask: np.ndarray):
    """Classify each [BLK, BLK] block; return (cls, idx, unique_blocks).

    unique_blocks[i] holds a transposed mask block pre-scaled by sqrt(hd) so
    the on-device exp((raw_scores + m') * 1/sqrt(hd)) equals the reference
    exp(raw_scores/sqrt(hd) + m).
    """
    cls = np.empty((NB, NB), dtype=np.int64)
    idx = np.full((NB, NB), -1, dtype=np.int64)
    uniq = []
    seen = {}
    for qi in range(NB):
        for kb in range(NB):
            blkm = mask[qi * BLK:(qi + 1) * BLK, kb * BLK:(kb + 1) * BLK]
            if np.all(blkm <= -1e8):
                cls[qi, kb] = SKIP
            elif not np.any(blkm):
                cls[qi, kb] = ZERO
            else:
                cls[qi, kb] = GENERAL
                key = blkm.tobytes()
                if key not in seen:
                    seen[key] = len(uniq)
                    uniq.append(np.ascontiguousarray(blkm.T) / SCALE)
                idx[qi, kb] = seen[key]
    if not uniq:
        uniq.append(np.zeros((BLK, BLK), dtype=np.float32))
    ublk = np.stack(uniq).astype(np.float32)
    return cls, idx, ublk


def build_program(cls, idx, n_ublk, iters=1, compute=COMPUTE, phases=("proj", "attn", "out")):
    DT = BF16 if compute == "bf16" else F32
    nc = bacc.Bacc("TRN2", target_bir_lowering=False, debug=False,
                   num_devices=N_CORES)

    xP = nc.dram_tensor("xP", [BLK, NCH, NB, CHUNK], DT, kind="ExternalInput").ap()
    wqP = nc.dram_tensor("wqP", [BLK, NB, JD], DT, kind="ExternalInput").ap()
    wkP = nc.dram_tensor("wkP", [BLK, NB, KVD], DT, kind="ExternalInput").ap()
    wvP = nc.dram_tensor("wvP", [BLK, NB, KVD], DT, kind="ExternalInput").ap()
    woP = nc.dram_tensor("woP", [BLK, NJT, D], DT, kind="ExternalInput").ap()
    identP = nc.dram_tensor("identP", [BLK, HEAD_DIM], DT, kind="ExternalInput").ap()
    maskP = nc.dram_tensor("maskP", [BLK, n_ublk, 2, BLK], DT, kind="ExternalInput").ap()
    identFP = nc.dram_tensor("identFP", [BLK, BLK], DT, kind="ExternalInput").ap()
    outD = nc.dram_tensor("out", [S, D], BF16, kind="ExternalOutput").ap()

    with tile.TileContext(nc) as tc:
        with (
            tc.tile_pool(name="wp", bufs=1) as wp,       # resident weights/consts
            tc.tile_pool(name="kvp", bufs=1) as kvp,     # resident KT/V
            tc.tile_pool(name="xp", bufs=2) as xp,       # streaming x chunks
            tc.tile_pool(name="qp", bufs=2) as qp,       # QT pairs
            tc.tile_pool(name="pp", bufs=int(OPTS.get("ppbufs", 6))) as pp,  # P pairs
            tc.tile_pool(name="mp", bufs=2) as mp,       # misc small
            tc.tile_pool(name="op", bufs=2) as op,       # out staging
            tc.tile_pool(name="psS", bufs=2, space="PSUM") as psS,   # stp pairs
            tc.tile_pool(name="psJ", bufs=1, space="PSUM") as psJ,   # proj/outproj/vps
            tc.tile_pool(name="psV", bufs=1, space="PSUM") as psV,   # pv pairs / kv proj
        ):
            def body():
                # ---- resident tiles ----
                wq_all = wp.tile([BLK, NB, JD], DT, tag="wq", name="wq_all")
                wk_all = wp.tile([BLK, NB, KVD], DT, tag="wk", name="wk_all")
                wv_all = wp.tile([BLK, NB, KVD], DT, tag="wv", name="wv_all")
                wo_all = wp.tile([BLK, NJT, D], DT, tag="wo", name="wo_all")
                ident = wp.tile([BLK, HEAD_DIM], DT, tag="ident", name="ident")
                identF = wp.tile([BLK, BLK], DT, tag="identF", name="identF")
                mk_all = wp.tile([BLK, n_ublk, 2, BLK], DT, tag="mk", name="mk_all")
                # wq in halves so the first Q pass can start sooner; wo last
                # (first needed at chunk 1's outproj interleave). x chunk 0 is
                # emitted before these (see below) so it wins HWDGE arbitration.
                def emit_w_dmas():
                    nc.scalar.dma_start(wq_all[:, :, 0:JD // 2],
                                        wqP[:, :, 0:JD // 2])
                    nc.scalar.dma_start(wq_all[:, :, JD // 2:JD],
                                        wqP[:, :, JD // 2:JD])
                    nc.scalar.dma_start(wk_all[:, :, :], wkP)
                    nc.scalar.dma_start(wv_all[:, :, :], wvP)
                    nc.scalar.dma_start(ident[:, :], identP)
                    nc.scalar.dma_start(identF[:, :], identFP)
                    nc.scalar.dma_start(mk_all[:, :, :, :], maskP)
                    nc.scalar.dma_start(wo_all[:, :, :], woP)

                # KT_all rows 0:64 hold K^T; rows 64:128 are a duplicate so
                # the sub=1 matmul's lhsT/rhs partition bases can match.
                KT_all = kvp.tile([BLK, KV_LOC, S], DT, tag="kt", name="KT_all")
                V_sb = [[kvp.tile([BLK, HEAD_DIM + 1], DT, tag=f"v{kv}_{kb}",
                                  name=f"v{kv}_{kb}")
                         for kb in range(NB)] for kv in range(KV_LOC)]

                xt_t = [None] * NCH
                qt_next = [None, None]   # jt-pair tiles, ping-pong via bufs=2

                def emit_x_dma(c, split=False):
                    xt = xp.tile([BLK, NB, CHUNK], DT, tag="xt", name="xt")
                    if split:
                        # halves: the first Q-pass matmul (db 0) unblocks after
                        # the first half lands.
                        nc.sync.dma_start(xt[:, 0:NB // 2, :], xP[:, c, 0:NB // 2, :])
                        nc.sync.dma_start(xt[:, NB // 2:NB, :], xP[:, c, NB // 2:NB, :])
                    else:
                        nc.sync.dma_start(xt[:, :, :], xP[:, c, :, :])
                    xt_t[c] = xt

                def emit_projQ(c, jp):
                    qt_ps = psJ.tile([BLK, 2, CHUNK], F32, tag="pj", name="qt_ps")
                    xt = xt_t[c]
                    for db in range(NB):
                        st_, sp_ = (db == 0), (db == NB - 1)
                        for jh in range(2):
                            jt = jp * 2 + jh
                            nc.tensor.matmul(
                                qt_ps[:, jh, :],
                                wq_all[:, db, jt * BLK:(jt + 1) * BLK],
                                xt[:, db, :], start=st_, stop=sp_)
                    qsb = qp.tile([BLK, 2, CHUNK], DT, tag=f"qt{jp}", name=f"qt{jp}")
                    if OPTS.get("qtsplit"):
                        for jh in range(2):
                            nc.vector.tensor_copy(qsb[:, jh, :], qt_ps[:, jh, :])
                    else:
                        nc.vector.tensor_copy(qsb[:, :, :], qt_ps[:, :, :])
                    qt_next[jp] = qsb

                def emit_projKV(c):
                    kv_ps = psV.tile([BLK, 2, CHUNK], F32, tag="pv", name="kv_ps")
                    xt = xt_t[c]
                    for db in range(NB):
                        st_, sp_ = (db == 0), (db == NB - 1)
                        nc.tensor.matmul(kv_ps[:, 0, :], wk_all[:, db, :],
                                         xt[:, db, :], start=st_, stop=sp_)
                        nc.tensor.matmul(kv_ps[:, 1, :], wv_all[:, db, :],
                                         xt[:, db, :], start=st_, stop=sp_)
                    sl = slice(c * CHUNK, (c + 1) * CHUNK)
                    for kv in range(KV_LOC):
                        nc.vector.tensor_copy(
                            KT_all[0:HEAD_DIM, kv, sl],
                            kv_ps[kv * HEAD_DIM:(kv + 1) * HEAD_DIM, 0, :])
                    nc.sync.dma_start(KT_all[HEAD_DIM:2 * HEAD_DIM, :, sl],
                                      KT_all[0:HEAD_DIM, :, sl])
                    vt_stage = mp.tile([BLK, CHUNK], DT, tag="vt", name="vt_stage")
                    nc.vector.tensor_copy(vt_stage[:, :], kv_ps[:, 1, :])
                    for kv in range(KV_LOC):
                        r0 = kv * HEAD_DIM
                        for kk in range(CHUNK // BLK):
                            kb = c * (CHUNK // BLK) + kk
                            v_ps = psJ.tile([BLK, HEAD_DIM], DT, tag="pj", name="v_ps")
                            nc.tensor.transpose(
                                v_ps[:, :],
                                vt_stage[r0:r0 + HEAD_DIM, kk * BLK:(kk + 1) * BLK],
                                ident[r0:r0 + HEAD_DIM, 0:HEAD_DIM])
                            nc.vector.tensor_copy(V_sb[kv][kb][:, 0:HEAD_DIM],
                                                  v_ps[:, :])
                            nc.vector.memset(V_sb[kv][kb][:, HEAD_DIM:HEAD_DIM + 1],
                                             1.0)

                filler = []      # FIFO of deferred outproj emission steps

                def pop_filler(n=1):
                    for _ in range(n):
                        if filler:
                            filler.pop(0)()

                def emit_attn_hp(c, hp, qis, qt_cur, mult_engine):
                    kv = hp // 2
                    jp, jh = hp // 2, hp % 2
                    kbs = [kb for kb in range(NB)
                           if any(cls[qi, kb] != SKIP for qi in qis)]
                    pv_ps = psV.tile([BLK, 2, CHUNK], F32, tag="pv", name="pv_ps")

                    def emit_scores(n_kb, kb):
                        nsk = [bool(cls[qi, kb] != SKIP) for qi in qis]
                        first = nsk.index(True)
                        if n_kb > 0 and all(nsk[first:]):
                            off = first * BLK  # valid blocks are a suffix
                        else:
                            off = 0
                        stp = psS.tile([BLK, 2, CHUNK], F32, tag="sp", name="stp")
                        for sub in range(2):
                            jr = sub * HEAD_DIM
                            nc.tensor.matmul(
                                stp[:, sub, off:],
                                KT_all[jr:jr + HEAD_DIM, kv, kb * BLK:(kb + 1) * BLK],
                                qt_cur[jp][jr:jr + HEAD_DIM, jh, off:],
                                start=True, stop=True)
                        # mask add ON THE PE: accumulate I.T @ mask into the
                        # score psum — avoids a DVE hop in the scores->exp->PV
                        # chain (f32r identity matmul, 1 cycle/row).
                        for ql in range(off // BLK, len(qis)):
                            qi = qis[ql]
                            if cls[qi, kb] == GENERAL:
                                csl = slice(ql * BLK, (ql + 1) * BLK)
                                for sub in range(2):
                                    nc.tensor.matmul(
                                        stp[:, sub, csl], identF[:, :],
                                        mk_all[:, idx[qi, kb], sub, :],
                                        start=False, stop=True,
                                        skip_group_check=True)
                        p = pp.tile([BLK, 2, CHUNK], DT, tag="p", name="p")
                        nc.scalar.activation(
                            p[:, :, off:], stp[:, :, off:],
                            mybir.ActivationFunctionType.Exp, scale=SCALE)
                        for ql in range(off // BLK, len(qis)):
                            if cls[qis[ql], kb] == SKIP:
                                nc.vector.memset(
                                    p[:, :, ql * BLK:(ql + 1) * BLK], 0.0)
                        return p, off

                    def emit_pv(n_kb, kb, p, off):
                        for sub in range(2):
                            nc.tensor.matmul(
                                pv_ps[0:HEAD_DIM + 1, sub, off:],
                                V_sb[kv][kb][:, :], p[:, sub, off:],
                                start=(n_kb == 0), stop=(n_kb == len(kbs) - 1))

                    # Software-pipelined: PV lags scores by `pvlag` kbs (PV only
                    # needs the SBUF p tile, so lag is free in PSUM) — hides the
                    # ACT latency + semaphore hops; filler steps (prev-chunk
                    # outproj) absorb the ACT-rate deficit.
                    lag = int(OPTS.get("pvlag", 2))
                    pop_filler()
                    pending = []
                    for n_kb, kb in enumerate(kbs):
                        p, off = emit_scores(n_kb, kb)
                        pending.append((n_kb, kb, p, off))
                        if len(pending) > lag:
                            emit_pv(*pending.pop(0))
                            pop_filler()
                    for pr in pending:
                        emit_pv(*pr)
                    # Copy PV out of PSUM immediately: frees the pv bank for the
                    # next hp-run ~4us earlier than running the whole normalize
                    # chain out of PSUM would.
                    pvs = mp.tile([HEAD_DIM + 1, 2, CHUNK], F32, tag="pvs",
                                  name="pvs")
                    nc.vector.tensor_copy(pvs[:, :, :], pv_ps[0:HEAD_DIM + 1, :, :])
                    recip = mp.tile([1, 2, CHUNK], F32, tag="recip", name="recip")
                    nc.vector.reciprocal(recip[:, :, :],
                                         pvs[HEAD_DIM:HEAD_DIM + 1, :, :])
                    bc = mp.tile([HEAD_DIM, 2, CHUNK], F32, tag="bc", name="bc")
                    nc.gpsimd.partition_broadcast(bc[:, :, :], recip[:, :, :])
                    aT = mp.tile([BLK, CHUNK], DT, tag=f"attnT{hp}",
                                 name=f"attnT{hp}")
                    for sub in range(2):
                        jr = sub * HEAD_DIM
                        mult_engine.tensor_tensor(
                            out=aT[jr:jr + HEAD_DIM, :],
                            in0=pvs[0:HEAD_DIM, sub, :], in1=bc[:, sub, :],
                            op=mybir.AluOpType.mult)
                    return aT

                def emit_outproj(qi, aTs, pools=None, defer=False):
                    ql = qi % (CHUNK // BLK)
                    state = {}

                    def st_alloc():
                        state["o"] = op.tile([BLK, 2, 2, CHUNK], BF16, tag="o",
                                             name="o_big")

                    def st_mm(eh, jt, pool, ptag):
                        if jt == 0:
                            state["acc"] = pool.tile([BLK, 2, CHUNK], F32,
                                                     tag=ptag, name="acc")
                        for ei in range(2):
                            et = eh * 2 + ei
                            nc.tensor.matmul(
                                state["acc"][:, ei, :],
                                aTs[jt][:, ql * BLK:(ql + 1) * BLK],
                                wo_all[:, jt, et * CHUNK:(et + 1) * CHUNK],
                                start=(jt == 0), stop=(jt == NJT - 1))

                    def st_copy(eh, split=False):
                        if split:
                            # halve slot turnaround: DVE + (otherwise idle) ACT
                            nc.vector.tensor_copy(state["o"][:, eh, 0, :],
                                                  state["acc"][:, 0, :])
                            nc.scalar.copy(state["o"][:, eh, 1, :],
                                           state["acc"][:, 1, :])
                        elif OPTS.get("ocopy") == "scalar":
                            nc.scalar.copy(state["o"][:, eh, :, :],
                                           state["acc"][:, :, :])
                        else:
                            nc.vector.tensor_copy(state["o"][:, eh, :, :],
                                                  state["acc"][:, :, :])

                    def st_dma():
                        nc.sync.dma_start(outD[qi * BLK:(qi + 1) * BLK, :],
                                          state["o"][:, :, :, :])

                    split = bool(pools)
                    steps = [st_alloc]
                    for eh in range(2):
                        if pools is None:
                            pool, ptag = psJ, "pj"
                        else:
                            pool, ptag = pools[eh % len(pools)]
                        for jt in range(NJT):
                            steps.append(lambda eh=eh, jt=jt, pl=pool, pt=ptag:
                                         st_mm(eh, jt, pl, pt))
                        steps.append(lambda eh=eh: st_copy(eh, split))
                        if defer:
                            # spacers: give the copy time to free the psum slot
                            # before the next acc's first matmul issues on PE
                            steps.extend([lambda: None] *
                                         int(OPTS.get("spacers", 2)))
                    steps.append(st_dma)
                    if defer:
                        filler.extend(steps)
                    else:
                        for s in steps:
                            s()

                # ---- prologue: chunk 0 projections ----
                emit_x_dma(0, split=True)
                emit_w_dmas()
                if "proj" in phases:
                    emit_projQ(0, 0)
                    emit_projQ(0, 1)
                    emit_projKV(0)

                prev_aTs = None
                prev_qis = None
                for c in range(NCH):
                    qis = list(range(c * (CHUNK // BLK), (c + 1) * (CHUNK // BLK)))
                    qt_cur = list(qt_next)
                    if c + 1 < NCH:
                        emit_x_dma(c + 1)
                    # gpsimd tensor_tensor measured ~7x slower than modeled
                    mult_eng = (nc.gpsimd if OPTS.get("mult") == "gpsimd"
                                else nc.vector)
                    block_mode = OPTS.get("ojmode", "filler") == "block"
                    rrb = [(psS, "sp"), (psJ, "pj")]
                    aTs = []
                    for hp in range(H_LOC // 2):
                        if "attn" in phases:
                            if "out" in phases and prev_aTs is not None:
                                if block_mode:
                                    if hp == 2:
                                        for i, qi in enumerate(prev_qis):
                                            emit_outproj(qi, prev_aTs,
                                                         pools=[rrb[i % 2],
                                                                rrb[(i + 1) % 2]])
                                else:
                                    emit_outproj(prev_qis[hp], prev_aTs,
                                                 defer=True)
                            aTs.append(emit_attn_hp(c, hp, qis, qt_cur, mult_eng))
                        if "proj" in phases and c + 1 < NCH:
                            if hp == 1:
                                emit_projQ(c + 1, 0)
                            elif hp == 2:
                                emit_projQ(c + 1, 1)
                            elif hp == 3:
                                emit_projKV(c + 1)
                    pop_filler(len(filler))   # drain before attnT ring reuse
                    prev_aTs, prev_qis = aTs, qis

                # epilogue: last chunk's outproj — attention psum is free, so
                # round-robin accs across all tags to pipeline the copies.
                if "attn" in phases and "out" in phases and prev_aTs is not None:
                    rr = [[(psS, "sp"), (psV, "pv")], [(psJ, "pj"), (psS, "sp")]]
                    for i, qi in enumerate(prev_qis):
                        emit_outproj(qi, prev_aTs, pools=rr[i % 2])

            if iters == 1:
                body()
            else:
                hints = (mybir.EngineType.PE, mybir.EngineType.DVE,
                         mybir.EngineType.Activation, mybir.EngineType.SP,
                         mybir.EngineType.Pool)
                unroll = int(OPTS.get("unroll", 1))
                if unroll > 1 and iters % unroll == 0:
                    with tc.For_i(0, iters // unroll, hint_engines=hints):
                        for _ in range(unroll):
                            body()
                else:
                    with tc.For_i(0, iters, hint_engines=hints):
                        body()
    nc.compile()
    return nc


def make_in_maps(x, wq, wk, wv, wo, ublk, compute=COMPUTE):
    npdt = ml_dtypes.bfloat16 if compute == "bf16" else np.float32
    ident = np.tile(np.eye(HEAD_DIM, dtype=np.float32), (2, 1)).astype(npdt)
    identf = np.eye(BLK, dtype=np.float32).astype(npdt)
    n_ublk = len(ublk)
    # maskP [128, n_ublk, 2, 128]: each transposed+prescaled block duplicated
    mk = np.repeat(ublk[:, None, :, :], 2, axis=1)          # [n, 2, 128, 128]
    mk = np.ascontiguousarray(mk.transpose(2, 0, 1, 3))     # [128, n, 2, 128]
    in_maps = []
    for cc in range(N_CORES):
        b, g = cc // TP, cc % TP
        xb = x[b]                                           # [S, D]
        # xP[p, c, db, j] = x[c*CHUNK+j, db*BLK+p]
        xp = xb.reshape(NCH, CHUNK, NB, BLK).transpose(3, 0, 2, 1)
        wql = wq[g * JD:(g + 1) * JD, :]                    # [JD, D]
        # wqP[p, db, jd] = wq[g*JD+jd, db*BLK+p]
        wqp = wql.reshape(JD, NB, BLK).transpose(2, 1, 0)
        wkl = wk[g * KVD:(g + 1) * KVD, :]
        wkp = wkl.reshape(KVD, NB, BLK).transpose(2, 1, 0)
        wvl = wv[g * KVD:(g + 1) * KVD, :]
        wvp = wvl.reshape(KVD, NB, BLK).transpose(2, 1, 0)
        wol = wo[:, g * JD:(g + 1) * JD]                    # [D, JD]
        # woP[p, jt, e] = wo[e, g*JD + jt*BLK + p]
        wop = wol.reshape(D, NJT, BLK).transpose(2, 1, 0)
        in_maps.append({
            "xP": np.ascontiguousarray(xp).astype(npdt),
            "wqP": np.ascontiguousarray(wqp).astype(npdt),
            "wkP": np.ascontiguousarray(wkp).astype(npdt),
            "wvP": np.ascontiguousarray(wvp).astype(npdt),
            "woP": np.ascontiguousarray(wop).astype(npdt),
            "identP": ident,
            "identFP": identf,
            "maskP": mk.astype(npdt),
        })
    return in_maps


def kernel(x, wq, wk, wv, wo, mask, start_pos):
    x = np.asarray(x, dtype=np.float32)
    wq = np.asarray(wq, dtype=np.float32)
    wk = np.asarray(wk, dtype=np.float32)
    wv = np.asarray(wv, dtype=np.float32)
    wo = np.asarray(wo, dtype=np.float32)
    mask = np.asarray(mask, dtype=np.float32)

    cls, idx, ublk = classify_mask(mask)
    nc = build_program(cls, idx, len(ublk), iters=1)
    in_maps = make_in_maps(x, wq, wk, wv, wo, ublk)
    res = run_bass_kernel_spmd(nc, in_maps, core_ids=list(range(N_CORES)),
                               trace=False)
    out = np.zeros((B, S, D), dtype=np.float32)
    for c in range(N_CORES):
        out[c // TP] += res.results[c]["out"].astype(np.float32)
    return out


# revision 46
# speedup vs baseline: 1.0780x; 1.0111x over previous
"""Trainium2 Bass kernel for GQA attention (prefill), SPMD over 8 NeuronCores.

Sharding: tensor-parallel over heads (4-way) x data-parallel over batch (2-way).
Core c handles batch c//4 and head-group c%4 (8 q-heads / 2 kv-heads of the
32/8 global heads). Each core computes a full [S, D] partial of the output
projection (wo row-parallel); the 4 partials per batch are summed on host
during unsharding.

v2 layout notes (vs the v1 baseline):
- All DRAM inputs are host-packed so each logical tensor loads with ONE
  DMA (weights) or one DMA per q-chunk (x): the shared HWDGE descriptor
  unit serializes DMA issue at ~625ns each, so 190 DMAs/iter was ~119us
  of hidden serialization. v2 issues ~30.
- Scores for the two heads of a jt-pair go into one 2-bank PSUM tile
  [128, 2, 512]; softmax exp is ONE activation op per (hp, kb) instead of
  two+, halving the ~150ns/op fixed overhead count on the ACT engine.
- PSUM is budgeted exactly: 2x stp pairs (4 banks) + 1 pv pair (2 banks)
  + 1 shared proj/outproj/transpose slot (2 banks) = 8 banks.
- The output projection of chunk c-1 and the QKV projection of chunk c+1
  are emitted interleaved with attention of chunk c so the PE always has
  independent work while ACT catches up on exps / DVE normalizes.
- Output is written bf16 (host upcasts + sums partials in f32).

The [S, S] additive mask is handled by classifying each 128x128 block on
host (SKIP / ZERO / GENERAL as in v1); GENERAL blocks ship a transposed,
pre-scaled copy duplicated x2 so one DVE add covers both heads of a pair.
"""

import numpy as np
import ml_dtypes

import concourse.bacc as bacc
import concourse.mybir as mybir
import concourse.tile as tile
from concourse.bass_utils import run_bass_kernel_spmd

# Problem shape (hardcoded per contract).
B, S, D = 2, 2048, 2048
N_HEADS, N_KV_HEADS, HEAD_DIM = 32, 8, 64
TP = 4            # head-group shards
N_CORES = 8
BLK = 128         # block size (partitions)
NB = S // BLK     # 16 blocks along seq
CHUNK = 512       # q-chunk (moving operand width)
NCH = S // CHUNK  # 4 q-chunks
H_LOC = N_HEADS // TP        # 8 q heads per core
KV_LOC = N_KV_HEADS // TP    # 2 kv heads per core
KVD = KV_LOC * HEAD_DIM      # 128
JD = H_LOC * HEAD_DIM        # 512 local head dims
NJT = JD // BLK              # 4 jt tiles
SCALE = 1.0 / float(np.sqrt(HEAD_DIM))

F32 = mybir.dt.float32
BF16 = mybir.dt.bfloat16

COMPUTE = "bf16"

# mask block classes
SKIP, ZERO, GENERAL = 0, 1, 2

# tuning knobs (see compare.py); defaults tuned on HW
OPTS = {"ojmode": "block"}


def classify_mask(mask: np.ndarray):
    """Classify each [BLK, BLK] block; return (cls, idx, unique_blocks).

    unique_blocks[i] holds a transposed mask block pre-scaled by sqrt(hd) so
    the on-device exp((raw_scores + m') * 1/sqrt(hd)) equals the reference
    exp(raw_scores/sqrt(hd) + m).
    """
    cls = np.empty((NB, NB), dtype=np.int64)
    idx = np.full((NB, NB), -1, dtype=np.int64)
    uniq = []
    seen = {}
    for qi in range(NB):
        for kb in range(NB):
            blkm = mask[qi * BLK:(qi + 1) * BLK, kb * BLK:(kb + 1) * BLK]
            if np.all(blkm <= -1e8):
                cls[qi, kb] = SKIP
            elif not np.any(blkm):
                cls[qi, kb] = ZERO
            else:
                cls[qi, kb] = GENERAL
                key = blkm.tobytes()
                if key not in seen:
                    seen[key] = len(uniq)
                    uniq.append(np.ascontiguousarray(blkm.T) / SCALE)
                idx[qi, kb] = seen[key]
    if not uniq:
        uniq.append(np.zeros((BLK, BLK), dtype=np.float32))
    ublk = np.stack(uniq).astype(np.float32)
    return cls, idx, ublk


def build_program(cls, idx, n_ublk, iters=1, compute=COMPUTE, phases=("proj", "attn", "out")):
    DT = BF16 if compute == "bf16" else F32
    nc = bacc.Bacc("TRN2", target_bir_lowering=False, debug=False,
                   num_devices=N_CORES)

    xP = nc.dram_tensor("xP", [BLK, NCH, NB, CHUNK], DT, kind="ExternalInput").ap()
    wqP = nc.dram_tensor("wqP", [BLK, NB, JD], DT, kind="ExternalInput").ap()
    wkP = nc.dram_tensor("wkP", [BLK, NB, KVD], DT, kind="ExternalInput").ap()
    wvP = nc.dram_tensor("wvP", [BLK, NB, KVD], DT, kind="ExternalInput").ap()
    woP = nc.dram_tensor("woP", [BLK, NJT, D], DT, kind="ExternalInput").ap()
    identP = nc.dram_tensor("identP", [BLK, HEAD_DIM], DT, kind="ExternalInput").ap()
    maskP = nc.dram_tensor("maskP", [BLK, n_ublk, 2, BLK], DT, kind="ExternalInput").ap()
    identFP = nc.dram_tensor("identFP", [BLK, BLK], DT, kind="ExternalInput").ap()
    outD = nc.dram_tensor("out", [S, D], BF16, kind="ExternalOutput").ap()

    with tile.TileContext(nc) as tc:
        with (
            tc.tile_pool(name="wp", bufs=1) as wp,       # resident weights/consts
            tc.tile_pool(name="kvp", bufs=1) as kvp,     # resident KT/V
            tc.tile_pool(name="xp", bufs=2) as xp,       # streaming x chunks
            tc.tile_pool(name="qp", bufs=2) as qp,       # QT pairs
            tc.tile_pool(name="pp", bufs=int(OPTS.get("ppbufs", 6))) as pp,  # P pairs
            tc.tile_pool(name="mp", bufs=2) as mp,       # misc small
            tc.tile_pool(name="op", bufs=2) as op,       # out staging
            tc.tile_pool(name="psS", bufs=2, space="PSUM") as psS,   # stp pairs
            tc.tile_pool(name="psJ", bufs=1, space="PSUM") as psJ,   # proj/outproj/vps
            tc.tile_pool(name="psV", bufs=1, space="PSUM") as psV,   # pv pairs / kv proj
        ):
            def body():
                # ---- resident tiles ----
                wq_all = wp.tile([BLK, NB, JD], DT, tag="wq", name="wq_all")
                wk_all = wp.tile([BLK, NB, KVD], DT, tag="wk", name="wk_all")
                wv_all = wp.tile([BLK, NB, KVD], DT, tag="wv", name="wv_all")
                wo_all = wp.tile([BLK, NJT, D], DT, tag="wo", name="wo_all")
                ident = wp.tile([BLK, HEAD_DIM], DT, tag="ident", name="ident")
                identF = wp.tile([BLK, BLK], DT, tag="identF", name="identF")
                mk_all = wp.tile([BLK, n_ublk, 2, BLK], DT, tag="mk", name="mk_all")
                # wq in halves so the first Q pass can start sooner; wo last
                # (first needed at chunk 1's outproj interleave). x chunk 0 is
                # emitted before these (see below) so it wins HWDGE arbitration.
                def emit_w_dmas():
                    nc.scalar.dma_start(wq_all[:, :, 0:JD // 2],
                                        wqP[:, :, 0:JD // 2])
                    nc.scalar.dma_start(wq_all[:, :, JD // 2:JD],
                                        wqP[:, :, JD // 2:JD])
                    nc.scalar.dma_start(wk_all[:, :, :], wkP)
                    nc.scalar.dma_start(wv_all[:, :, :], wvP)
                    nc.scalar.dma_start(ident[:, :], identP)
                    nc.scalar.dma_start(identF[:, :], identFP)
                    nc.scalar.dma_start(mk_all[:, :, :, :], maskP)
                    nc.scalar.dma_start(wo_all[:, :, :], woP)

                # KT_all rows 0:64 hold K^T; rows 64:128 are a duplicate so
                # the sub=1 matmul's lhsT/rhs partition bases can match.
                KT_all = kvp.tile([BLK, KV_LOC, S], DT, tag="kt", name="KT_all")
                V_sb = [[kvp.tile([BLK, HEAD_DIM + 1], DT, tag=f"v{kv}_{kb}",
                                  name=f"v{kv}_{kb}")
                         for kb in range(NB)] for kv in range(KV_LOC)]

                xt_t = [None] * NCH
                qt_next = [None, None]   # jt-pair tiles, ping-pong via bufs=2

                def emit_x_dma(c, split=False):
                    xt = xp.tile([BLK, NB, CHUNK], DT, tag="xt", name="xt")
                    if split:
                        # halves: the first Q-pass matmul (db 0) unblocks after
                        # the first half lands.
                        nc.sync.dma_start(xt[:, 0:NB // 2, :], xP[:, c, 0:NB // 2, :])
                        nc.sync.dma_start(xt[:, NB // 2:NB, :], xP[:, c, NB // 2:NB, :])
                    else:
                        nc.sync.dma_start(xt[:, :, :], xP[:, c, :, :])
                    xt_t[c] = xt

                def emit_projQ(c, jp, defer=False):
                    state = {}

                    def st_db(db):
                        if db == 0:
                            state["ps"] = psJ.tile([BLK, 2, CHUNK], F32,
                                                   tag="pj", name="qt_ps")
                        xt = xt_t[c]
                        st_, sp_ = (db == 0), (db == NB - 1)
                        for jh in range(2):
                            jt = jp * 2 + jh
                            nc.tensor.matmul(
                                state["ps"][:, jh, :],
                                wq_all[:, db, jt * BLK:(jt + 1) * BLK],
                                xt[:, db, :], start=st_, stop=sp_)

                    def st_copy():
                        qsb = qp.tile([BLK, 2, CHUNK], DT, tag=f"qt{jp}",
                                      name=f"qt{jp}")
                        if OPTS.get("qtsplit"):
                            for jh in range(2):
                                nc.vector.tensor_copy(qsb[:, jh, :],
                                                      state["ps"][:, jh, :])
                        else:
                            nc.vector.tensor_copy(qsb[:, :, :],
                                                  state["ps"][:, :, :])
                        qt_next[jp] = qsb

                    steps = [lambda db=db: st_db(db) for db in range(NB)]
                    steps.append(st_copy)
                    if defer:
                        filler.extend(steps)
                    else:
                        for s in steps:
                            s()

                def emit_projKV(c):
                    kv_ps = psV.tile([BLK, 2, CHUNK], F32, tag="pv", name="kv_ps")
                    xt = xt_t[c]
                    for db in range(NB):
                        st_, sp_ = (db == 0), (db == NB - 1)
                        nc.tensor.matmul(kv_ps[:, 0, :], wk_all[:, db, :],
                                         xt[:, db, :], start=st_, stop=sp_)
                        nc.tensor.matmul(kv_ps[:, 1, :], wv_all[:, db, :],
                                         xt[:, db, :], start=st_, stop=sp_)
                    sl = slice(c * CHUNK, (c + 1) * CHUNK)
                    for kv in range(KV_LOC):
                        nc.vector.tensor_copy(
                            KT_all[0:HEAD_DIM, kv, sl],
                            kv_ps[kv * HEAD_DIM:(kv + 1) * HEAD_DIM, 0, :])
                    nc.sync.dma_start(KT_all[HEAD_DIM:2 * HEAD_DIM, :, sl],
                                      KT_all[0:HEAD_DIM, :, sl])
                    vt_stage = mp.tile([BLK, CHUNK], DT, tag="vt", name="vt_stage")
                    nc.vector.tensor_copy(vt_stage[:, :], kv_ps[:, 1, :])
                    for kv in range(KV_LOC):
                        r0 = kv * HEAD_DIM
                        for kk in range(CHUNK // BLK):
                            kb = c * (CHUNK // BLK) + kk
                            v_ps = psJ.tile([BLK, HEAD_DIM], DT, tag="pj", name="v_ps")
                            nc.tensor.transpose(
                                v_ps[:, :],
                                vt_stage[r0:r0 + HEAD_DIM, kk * BLK:(kk + 1) * BLK],
                                ident[r0:r0 + HEAD_DIM, 0:HEAD_DIM])
                            nc.vector.tensor_copy(V_sb[kv][kb][:, 0:HEAD_DIM],
                                                  v_ps[:, :])
                            nc.vector.memset(V_sb[kv][kb][:, HEAD_DIM:HEAD_DIM + 1],
                                             1.0)

                filler = []      # FIFO of deferred outproj emission steps

                def pop_filler(n=1):
                    for _ in range(n):
                        if filler:
                            filler.pop(0)()

                def emit_attn_hp(c, hp, qis, qt_cur, mult_engine):
                    kv = hp // 2
                    jp, jh = hp // 2, hp % 2
                    kbs = [kb for kb in range(NB)
                           if any(cls[qi, kb] != SKIP for qi in qis)]
                    pv_ps = psV.tile([BLK, 2, CHUNK], F32, tag="pv", name="pv_ps")

                    def emit_scores(n_kb, kb):
                        nsk = [bool(cls[qi, kb] != SKIP) for qi in qis]
                        first = nsk.index(True)
                        if n_kb > 0 and all(nsk[first:]):
                            off = first * BLK  # valid blocks are a suffix
                        else:
                            off = 0
                        stp = psS.tile([BLK, 2, CHUNK], F32, tag="sp", name="stp")
                        for sub in range(2):
                            jr = sub * HEAD_DIM
                            nc.tensor.matmul(
                                stp[:, sub, off:],
                                KT_all[jr:jr + HEAD_DIM, kv, kb * BLK:(kb + 1) * BLK],
                                qt_cur[jp][jr:jr + HEAD_DIM, jh, off:],
                                start=True, stop=True)
                        # mask add ON THE PE: accumulate I.T @ mask into the
                        # score psum — avoids a DVE hop in the scores->exp->PV
                        # chain (f32r identity matmul, 1 cycle/row).
                        for ql in range(off // BLK, len(qis)):
                            qi = qis[ql]
                            if cls[qi, kb] == GENERAL:
                                csl = slice(ql * BLK, (ql + 1) * BLK)
                                for sub in range(2):
                                    nc.tensor.matmul(
                                        stp[:, sub, csl], identF[:, :],
                                        mk_all[:, idx[qi, kb], sub, :],
                                        start=False, stop=True,
                                        skip_group_check=True)
                        p = pp.tile([BLK, 2, CHUNK], DT, tag="p", name="p")
                        nc.scalar.activation(
                            p[:, :, off:], stp[:, :, off:],
                            mybir.ActivationFunctionType.Exp, scale=SCALE)
                        for ql in range(off // BLK, len(qis)):
                            if cls[qis[ql], kb] == SKIP:
                                nc.vector.memset(
                                    p[:, :, ql * BLK:(ql + 1) * BLK], 0.0)
                        return p, off

                    def emit_pv(n_kb, kb, p, off):
                        for sub in range(2):
                            nc.tensor.matmul(
                                pv_ps[0:HEAD_DIM + 1, sub, off:],
                                V_sb[kv][kb][:, :], p[:, sub, off:],
                                start=(n_kb == 0), stop=(n_kb == len(kbs) - 1))

                    # Software-pipelined: PV lags scores by `pvlag` kbs (PV only
                    # needs the SBUF p tile, so lag is free in PSUM) — hides the
                    # ACT latency + semaphore hops; filler steps (prev-chunk
                    # outproj) absorb the ACT-rate deficit.
                    lag = int(OPTS.get("pvlag", 2))
                    pop_filler()
                    pending = []
                    for n_kb, kb in enumerate(kbs):
                        p, off = emit_scores(n_kb, kb)
                        pending.append((n_kb, kb, p, off))
                        if len(pending) > lag:
                            emit_pv(*pending.pop(0))
                            pop_filler()
                    for pr in pending:
                        emit_pv(*pr)
                    # Copy PV out of PSUM immediately: frees the pv bank for the
                    # next hp-run ~4us earlier than running the whole normalize
                    # chain out of PSUM would.
                    pvs = mp.tile([HEAD_DIM + 1, 2, CHUNK], F32, tag="pvs",
                                  name="pvs")
                    nc.vector.tensor_copy(pvs[:, :, :], pv_ps[0:HEAD_DIM + 1, :, :])
                    recip = mp.tile([1, 2, CHUNK], F32, tag="recip", name="recip")
                    nc.vector.reciprocal(recip[:, :, :],
                                         pvs[HEAD_DIM:HEAD_DIM + 1, :, :])
                    bc = mp.tile([HEAD_DIM, 2, CHUNK], F32, tag="bc", name="bc")
                    nc.gpsimd.partition_broadcast(bc[:, :, :], recip[:, :, :])
                    aT = mp.tile([BLK, CHUNK], DT, tag=f"attnT{hp}",
                                 name=f"attnT{hp}")
                    for sub in range(2):
                        jr = sub * HEAD_DIM
                        mult_engine.tensor_tensor(
                            out=aT[jr:jr + HEAD_DIM, :],
                            in0=pvs[0:HEAD_DIM, sub, :], in1=bc[:, sub, :],
                            op=mybir.AluOpType.mult)
                    return aT

                def emit_outproj(qi, aTs, pools=None, defer=False):
                    ql = qi % (CHUNK // BLK)
                    state = {}

                    def st_alloc():
                        state["o"] = op.tile([BLK, 2, 2, CHUNK], BF16, tag="o",
                                             name="o_big")

                    def st_mm(eh, jt, pool, ptag):
                        if jt == 0:
                            state["acc"] = pool.tile([BLK, 2, CHUNK], F32,
                                                     tag=ptag, name="acc")
                        for ei in range(2):
                            et = eh * 2 + ei
                            nc.tensor.matmul(
                                state["acc"][:, ei, :],
                                aTs[jt][:, ql * BLK:(ql + 1) * BLK],
                                wo_all[:, jt, et * CHUNK:(et + 1) * CHUNK],
                                start=(jt == 0), stop=(jt == NJT - 1))

                    def st_copy(eh, split=False):
                        if split:
                            # halve slot turnaround: DVE + (otherwise idle) ACT
                            nc.vector.tensor_copy(state["o"][:, eh, 0, :],
                                                  state["acc"][:, 0, :])
                            nc.scalar.copy(state["o"][:, eh, 1, :],
                                           state["acc"][:, 1, :])
                        elif OPTS.get("ocopy") == "scalar":
                            nc.scalar.copy(state["o"][:, eh, :, :],
                                           state["acc"][:, :, :])
                        else:
                            nc.vector.tensor_copy(state["o"][:, eh, :, :],
                                                  state["acc"][:, :, :])

                    def st_dma():
                        nc.sync.dma_start(outD[qi * BLK:(qi + 1) * BLK, :],
                                          state["o"][:, :, :, :])

                    split = bool(pools)
                    steps = [st_alloc]
                    for eh in range(2):
                        if pools is None:
                            pool, ptag = psJ, "pj"
                        else:
                            pool, ptag = pools[eh % len(pools)]
                        for jt in range(NJT):
                            steps.append(lambda eh=eh, jt=jt, pl=pool, pt=ptag:
                                         st_mm(eh, jt, pl, pt))
                        steps.append(lambda eh=eh: st_copy(eh, split))
                        if defer:
                            # spacers: give the copy time to free the psum slot
                            # before the next acc's first matmul issues on PE
                            steps.extend([lambda: None] *
                                         int(OPTS.get("spacers", 2)))
                    steps.append(st_dma)
                    if defer:
                        filler.extend(steps)
                    else:
                        for s in steps:
                            s()

                # ---- prologue: chunk 0 projections ----
                emit_x_dma(0, split=True)
                emit_w_dmas()
                if "proj" in phases:
                    emit_projQ(0, 0)
                    emit_projQ(0, 1)
                    emit_projKV(0)

                prev_aTs = None
                prev_qis = None
                for c in range(NCH):
                    qis = list(range(c * (CHUNK // BLK), (c + 1) * (CHUNK // BLK)))
                    qt_cur = list(qt_next)
                    if c + 1 < NCH:
                        emit_x_dma(c + 1, split=True)
                    # gpsimd tensor_tensor measured ~7x slower than modeled
                    mult_eng = (nc.gpsimd if OPTS.get("mult") == "gpsimd"
                                else nc.vector)
                    block_mode = OPTS.get("ojmode", "filler") == "block"
                    projfill = bool(int(OPTS.get("projfill", 0)))
                    if projfill:
                        rrb = [(psS, "sp"), (psS, "sp")]
                    else:
                        rrb = [(psS, "sp"), (psJ, "pj")]
                    aTs = []
                    for hp in range(H_LOC // 2):
                        if "proj" in phases and c + 1 < NCH and projfill and hp == 1:
                            emit_projQ(c + 1, 0, defer=True)
                            emit_projQ(c + 1, 1, defer=True)
                        if "attn" in phases:
                            if "out" in phases and prev_aTs is not None:
                                if block_mode:
                                    if hp == 2:
                                        for i, qi in enumerate(prev_qis):
                                            emit_outproj(qi, prev_aTs,
                                                         pools=[rrb[i % 2],
                                                                rrb[(i + 1) % 2]])
                                else:
                                    emit_outproj(prev_qis[hp], prev_aTs,
                                                 defer=True)
                            aTs.append(emit_attn_hp(c, hp, qis, qt_cur, mult_eng))
                        if "proj" in phases and c + 1 < NCH and not projfill:
                            if hp == 1:
                                emit_projQ(c + 1, 0)
                            elif hp == 2:
                                emit_projQ(c + 1, 1)
                        if "proj" in phases and c + 1 < NCH and hp == 3:
                            emit_projKV(c + 1)
                    pop_filler(len(filler))   # drain before attnT ring reuse
                    prev_aTs, prev_qis = aTs, qis

                # epilogue: last chunk's outproj — attention psum is free, so
                # round-robin accs across all tags to pipeline the copies.
                if "attn" in phases and "out" in phases and prev_aTs is not None:
                    rr = [[(psS, "sp"), (psV, "pv")], [(psJ, "pj"), (psS, "sp")]]
                    for i, qi in enumerate(prev_qis):
                        emit_outproj(qi, prev_aTs, pools=rr[i % 2])

            if iters == 1:
                body()
            else:
                hints = (mybir.EngineType.PE, mybir.EngineType.DVE,
                         mybir.EngineType.Activation, mybir.EngineType.SP,
                         mybir.EngineType.Pool)
                unroll = int(OPTS.get("unroll", 1))
                if unroll > 1 and iters % unroll == 0:
                    with tc.For_i(0, iters // unroll, hint_engines=hints):
                        for _ in range(unroll):
                            body()
                else:
                    with tc.For_i(0, iters, hint_engines=hints):
                        body()
    nc.compile()
    return nc


def make_in_maps(x, wq, wk, wv, wo, ublk, compute=COMPUTE):
    npdt = ml_dtypes.bfloat16 if compute == "bf16" else np.float32
    ident = np.tile(np.eye(HEAD_DIM, dtype=np.float32), (2, 1)).astype(npdt)
    identf = np.eye(BLK, dtype=np.float32).astype(npdt)
    n_ublk = len(ublk)
    # maskP [128, n_ublk, 2, 128]: each transposed+prescaled block duplicated
    mk = np.repeat(ublk[:, None, :, :], 2, axis=1)          # [n, 2, 128, 128]
    mk = np.ascontiguousarray(mk.transpose(2, 0, 1, 3))     # [128, n, 2, 128]
    in_maps = []
    for cc in range(N_CORES):
        b, g = cc // TP, cc % TP
        xb = x[b]                                           # [S, D]
        # xP[p, c, db, j] = x[c*CHUNK+j, db*BLK+p]
        xp = xb.reshape(NCH, CHUNK, NB, BLK).transpose(3, 0, 2, 1)
        wql = wq[g * JD:(g + 1) * JD, :]                    # [JD, D]
        # wqP[p, db, jd] = wq[g*JD+jd, db*BLK+p]
        wqp = wql.reshape(JD, NB, BLK).transpose(2, 1, 0)
        wkl = wk[g * KVD:(g + 1) * KVD, :]
        wkp = wkl.reshape(KVD, NB, BLK).transpose(2, 1, 0)
        wvl = wv[g * KVD:(g + 1) * KVD, :]
        wvp = wvl.reshape(KVD, NB, BLK).transpose(2, 1, 0)
        wol = wo[:, g * JD:(g + 1) * JD]                    # [D, JD]
        # woP[p, jt, e] = wo[e, g*JD + jt*BLK + p]
        wop = wol.reshape(D, NJT, BLK).transpose(2, 1, 0)
        in_maps.append({
            "xP": np.ascontiguousarray(xp).astype(npdt),
            "wqP": np.ascontiguousarray(wqp).astype(npdt),
            "wkP": np.ascontiguousarray(wkp).astype(npdt),
            "wvP": np.ascontiguousarray(wvp).astype(npdt),
            "woP": np.ascontiguousarray(wop).astype(npdt),
            "identP": ident,
            "identFP": identf,
            "maskP": mk.astype(npdt),
        })
    return in_maps


def kernel(x, wq, wk, wv, wo, mask, start_pos):
    x = np.asarray(x, dtype=np.float32)
    wq = np.asarray(wq, dtype=np.float32)
    wk = np.asarray(wk, dtype=np.float32)
    wv = np.asarray(wv, dtype=np.float32)
    wo = np.asarray(wo, dtype=np.float32)
    mask = np.asarray(mask, dtype=np.float32)

    cls, idx, ublk = classify_mask(mask)
    nc = build_program(cls, idx, len(ublk), iters=1)
    in_maps = make_in_maps(x, wq, wk, wv, wo, ublk)
    res = run_bass_kernel_spmd(nc, in_maps, core_ids=list(range(N_CORES)),
                               trace=False)
    out = np.zeros((B, S, D), dtype=np.float32)
    for c in range(N_CORES):
        out[c // TP] += res.results[c]["out"].astype(np.float32)
    return out


# revision 47
# speedup vs baseline: 1.1017x; 1.0220x over previous
"""Trainium2 Bass kernel for GQA attention (prefill), SPMD over 8 NeuronCores.

Sharding: tensor-parallel over heads (4-way) x data-parallel over batch (2-way).
Core c handles batch c//4 and head-group c%4 (8 q-heads / 2 kv-heads of the
32/8 global heads). Each core computes a full [S, D] partial of the output
projection (wo row-parallel); the 4 partials per batch are summed on host
during unsharding.

v2 layout notes (vs the v1 baseline):
- All DRAM inputs are host-packed so each logical tensor loads with ONE
  DMA (weights) or one DMA per q-chunk (x): the shared HWDGE descriptor
  unit serializes DMA issue at ~625ns each, so 190 DMAs/iter was ~119us
  of hidden serialization. v2 issues ~30.
- Scores for the two heads of a jt-pair go into one 2-bank PSUM tile
  [128, 2, 512]; softmax exp is ONE activation op per (hp, kb) instead of
  two+, halving the ~150ns/op fixed overhead count on the ACT engine.
- PSUM is budgeted exactly: 2x stp pairs (4 banks) + 1 pv pair (2 banks)
  + 1 shared proj/outproj/transpose slot (2 banks) = 8 banks.
- The output projection of chunk c-1 and the QKV projection of chunk c+1
  are emitted interleaved with attention of chunk c so the PE always has
  independent work while ACT catches up on exps / DVE normalizes.
- Output is written bf16 (host upcasts + sums partials in f32).

The [S, S] additive mask is handled by classifying each 128x128 block on
host (SKIP / ZERO / GENERAL as in v1); GENERAL blocks ship a transposed,
pre-scaled copy duplicated x2 so one DVE add covers both heads of a pair.
"""

import numpy as np
import ml_dtypes

import concourse.bacc as bacc
import concourse.mybir as mybir
import concourse.tile as tile
from concourse.bass_utils import run_bass_kernel_spmd

# Problem shape (hardcoded per contract).
B, S, D = 2, 2048, 2048
N_HEADS, N_KV_HEADS, HEAD_DIM = 32, 8, 64
TP = 4            # head-group shards
N_CORES = 8
BLK = 128         # block size (partitions)
NB = S // BLK     # 16 blocks along seq
CHUNK = 512       # q-chunk (moving operand width)
NCH = S // CHUNK  # 4 q-chunks
H_LOC = N_HEADS // TP        # 8 q heads per core
KV_LOC = N_KV_HEADS // TP    # 2 kv heads per core
KVD = KV_LOC * HEAD_DIM      # 128
JD = H_LOC * HEAD_DIM        # 512 local head dims
NJT = JD // BLK              # 4 jt tiles
SCALE = 1.0 / float(np.sqrt(HEAD_DIM))

F32 = mybir.dt.float32
BF16 = mybir.dt.bfloat16

COMPUTE = "bf16"

# mask block classes
SKIP, ZERO, GENERAL = 0, 1, 2

# tuning knobs (see compare.py); defaults tuned on HW
OPTS = {"ojmode": "block", "pvlag": 3}


def classify_mask(mask: np.ndarray):
    """Classify each [BLK, BLK] block; return (cls, idx, unique_blocks).

    unique_blocks[i] holds a transposed mask block pre-scaled by sqrt(hd) so
    the on-device exp((raw_scores + m') * 1/sqrt(hd)) equals the reference
    exp(raw_scores/sqrt(hd) + m).
    """
    cls = np.empty((NB, NB), dtype=np.int64)
    idx = np.full((NB, NB), -1, dtype=np.int64)
    uniq = []
    seen = {}
    for qi in range(NB):
        for kb in range(NB):
            blkm = mask[qi * BLK:(qi + 1) * BLK, kb * BLK:(kb + 1) * BLK]
            if np.all(blkm <= -1e8):
                cls[qi, kb] = SKIP
            elif not np.any(blkm):
                cls[qi, kb] = ZERO
            else:
                cls[qi, kb] = GENERAL
                key = blkm.tobytes()
                if key not in seen:
                    seen[key] = len(uniq)
                    uniq.append(np.ascontiguousarray(blkm.T) / SCALE)
                idx[qi, kb] = seen[key]
    if not uniq:
        uniq.append(np.zeros((BLK, BLK), dtype=np.float32))
    ublk = np.stack(uniq).astype(np.float32)
    return cls, idx, ublk


def build_program(cls, idx, n_ublk, iters=1, compute=COMPUTE, phases=("proj", "attn", "out")):
    DT = BF16 if compute == "bf16" else F32
    nc = bacc.Bacc("TRN2", target_bir_lowering=False, debug=False,
                   num_devices=N_CORES)

    xP = nc.dram_tensor("xP", [BLK, NCH, NB, CHUNK], DT, kind="ExternalInput").ap()
    wqP = nc.dram_tensor("wqP", [BLK, NB, JD], DT, kind="ExternalInput").ap()
    wkP = nc.dram_tensor("wkP", [BLK, NB, KVD], DT, kind="ExternalInput").ap()
    wvP = nc.dram_tensor("wvP", [BLK, NB, KVD], DT, kind="ExternalInput").ap()
    woP = nc.dram_tensor("woP", [BLK, NJT, D], DT, kind="ExternalInput").ap()
    identP = nc.dram_tensor("identP", [BLK, HEAD_DIM], DT, kind="ExternalInput").ap()
    maskP = nc.dram_tensor("maskP", [BLK, n_ublk, 2, BLK], DT, kind="ExternalInput").ap()
    identFP = nc.dram_tensor("identFP", [BLK, BLK], DT, kind="ExternalInput").ap()
    outD = nc.dram_tensor("out", [S, D], BF16, kind="ExternalOutput").ap()

    with tile.TileContext(nc) as tc:
        with (
            tc.tile_pool(name="wp", bufs=1) as wp,       # resident weights/consts
            tc.tile_pool(name="kvp", bufs=1) as kvp,     # resident KT/V
            tc.tile_pool(name="xp", bufs=2) as xp,       # streaming x chunks
            tc.tile_pool(name="qp", bufs=2) as qp,       # QT pairs
            tc.tile_pool(name="pp", bufs=int(OPTS.get("ppbufs", 6))) as pp,  # P pairs
            tc.tile_pool(name="mp", bufs=2) as mp,       # misc small
            tc.tile_pool(name="op", bufs=2) as op,       # out staging
            tc.tile_pool(name="psS", bufs=2, space="PSUM") as psS,   # stp pairs
            tc.tile_pool(name="psJ", bufs=1, space="PSUM") as psJ,   # proj/outproj/vps
            tc.tile_pool(name="psV", bufs=1, space="PSUM") as psV,   # pv pairs / kv proj
        ):
            def body():
                # ---- resident tiles ----
                wq_all = wp.tile([BLK, NB, JD], DT, tag="wq", name="wq_all")
                wk_all = wp.tile([BLK, NB, KVD], DT, tag="wk", name="wk_all")
                wv_all = wp.tile([BLK, NB, KVD], DT, tag="wv", name="wv_all")
                wo_all = wp.tile([BLK, NJT, D], DT, tag="wo", name="wo_all")
                ident = wp.tile([BLK, HEAD_DIM], DT, tag="ident", name="ident")
                identF = wp.tile([BLK, BLK], DT, tag="identF", name="identF")
                mk_all = wp.tile([BLK, n_ublk, 2, BLK], DT, tag="mk", name="mk_all")
                # wq in halves so the first Q pass can start sooner; wo last
                # (first needed at chunk 1's outproj interleave). x chunk 0 is
                # emitted before these (see below) so it wins HWDGE arbitration.
                def emit_w_dmas():
                    nc.scalar.dma_start(wq_all[:, :, 0:JD // 2],
                                        wqP[:, :, 0:JD // 2])
                    nc.scalar.dma_start(wq_all[:, :, JD // 2:JD],
                                        wqP[:, :, JD // 2:JD])
                    nc.scalar.dma_start(wk_all[:, :, :], wkP)
                    nc.scalar.dma_start(wv_all[:, :, :], wvP)
                    nc.scalar.dma_start(ident[:, :], identP)
                    nc.scalar.dma_start(identF[:, :], identFP)
                    nc.scalar.dma_start(mk_all[:, :, :, :], maskP)
                    nc.scalar.dma_start(wo_all[:, :, :], woP)

                # KT_all rows 0:64 hold K^T; rows 64:128 are a duplicate so
                # the sub=1 matmul's lhsT/rhs partition bases can match.
                KT_all = kvp.tile([BLK, KV_LOC, S], DT, tag="kt", name="KT_all")
                V_sb = [[kvp.tile([BLK, HEAD_DIM + 1], DT, tag=f"v{kv}_{kb}",
                                  name=f"v{kv}_{kb}")
                         for kb in range(NB)] for kv in range(KV_LOC)]

                xt_t = [None] * NCH
                qt_next = [None, None]   # jt-pair tiles, ping-pong via bufs=2

                def emit_x_dma(c, split=False):
                    xt = xp.tile([BLK, NB, CHUNK], DT, tag="xt", name="xt")
                    if split:
                        # halves: the first Q-pass matmul (db 0) unblocks after
                        # the first half lands.
                        nc.sync.dma_start(xt[:, 0:NB // 2, :], xP[:, c, 0:NB // 2, :])
                        nc.sync.dma_start(xt[:, NB // 2:NB, :], xP[:, c, NB // 2:NB, :])
                    else:
                        nc.sync.dma_start(xt[:, :, :], xP[:, c, :, :])
                    xt_t[c] = xt

                def emit_projQ(c, jp, defer=False):
                    state = {}

                    def st_db(db):
                        if db == 0:
                            state["ps"] = psJ.tile([BLK, 2, CHUNK], F32,
                                                   tag="pj", name="qt_ps")
                        xt = xt_t[c]
                        st_, sp_ = (db == 0), (db == NB - 1)
                        for jh in range(2):
                            jt = jp * 2 + jh
                            nc.tensor.matmul(
                                state["ps"][:, jh, :],
                                wq_all[:, db, jt * BLK:(jt + 1) * BLK],
                                xt[:, db, :], start=st_, stop=sp_)

                    def st_copy():
                        qsb = qp.tile([BLK, 2, CHUNK], DT, tag=f"qt{jp}",
                                      name=f"qt{jp}")
                        if OPTS.get("qtsplit"):
                            for jh in range(2):
                                nc.vector.tensor_copy(qsb[:, jh, :],
                                                      state["ps"][:, jh, :])
                        else:
                            nc.vector.tensor_copy(qsb[:, :, :],
                                                  state["ps"][:, :, :])
                        qt_next[jp] = qsb

                    steps = [lambda db=db: st_db(db) for db in range(NB)]
                    steps.append(st_copy)
                    if defer:
                        filler.extend(steps)
                    else:
                        for s in steps:
                            s()

                def emit_projKV(c):
                    kv_ps = psV.tile([BLK, 2, CHUNK], F32, tag="pv", name="kv_ps")
                    xt = xt_t[c]
                    for db in range(NB):
                        st_, sp_ = (db == 0), (db == NB - 1)
                        nc.tensor.matmul(kv_ps[:, 0, :], wk_all[:, db, :],
                                         xt[:, db, :], start=st_, stop=sp_)
                        nc.tensor.matmul(kv_ps[:, 1, :], wv_all[:, db, :],
                                         xt[:, db, :], start=st_, stop=sp_)
                    sl = slice(c * CHUNK, (c + 1) * CHUNK)
                    for kv in range(KV_LOC):
                        nc.vector.tensor_copy(
                            KT_all[0:HEAD_DIM, kv, sl],
                            kv_ps[kv * HEAD_DIM:(kv + 1) * HEAD_DIM, 0, :])
                    nc.sync.dma_start(KT_all[HEAD_DIM:2 * HEAD_DIM, :, sl],
                                      KT_all[0:HEAD_DIM, :, sl])
                    vt_stage = mp.tile([BLK, CHUNK], DT, tag="vt", name="vt_stage")
                    nc.vector.tensor_copy(vt_stage[:, :], kv_ps[:, 1, :])
                    for kv in range(KV_LOC):
                        r0 = kv * HEAD_DIM
                        for kk in range(CHUNK // BLK):
                            kb = c * (CHUNK // BLK) + kk
                            v_ps = psJ.tile([BLK, HEAD_DIM], DT, tag="pj", name="v_ps")
                            nc.tensor.transpose(
                                v_ps[:, :],
                                vt_stage[r0:r0 + HEAD_DIM, kk * BLK:(kk + 1) * BLK],
                                ident[r0:r0 + HEAD_DIM, 0:HEAD_DIM])
                            nc.vector.tensor_copy(V_sb[kv][kb][:, 0:HEAD_DIM],
                                                  v_ps[:, :])
                            nc.vector.memset(V_sb[kv][kb][:, HEAD_DIM:HEAD_DIM + 1],
                                             1.0)

                filler = []      # FIFO of deferred outproj emission steps

                def pop_filler(n=1):
                    for _ in range(n):
                        if filler:
                            filler.pop(0)()

                def emit_attn_hp(c, hp, qis, qt_cur, mult_engine):
                    kv = hp // 2
                    jp, jh = hp // 2, hp % 2
                    kbs = [kb for kb in range(NB)
                           if any(cls[qi, kb] != SKIP for qi in qis)]
                    pv_ps = psV.tile([BLK, 2, CHUNK], F32, tag="pv", name="pv_ps")

                    def emit_scores(n_kb, kb):
                        nsk = [bool(cls[qi, kb] != SKIP) for qi in qis]
                        first = nsk.index(True)
                        if n_kb > 0 and all(nsk[first:]):
                            off = first * BLK  # valid blocks are a suffix
                        else:
                            off = 0
                        stp = psS.tile([BLK, 2, CHUNK], F32, tag="sp", name="stp")
                        for sub in range(2):
                            jr = sub * HEAD_DIM
                            nc.tensor.matmul(
                                stp[:, sub, off:],
                                KT_all[jr:jr + HEAD_DIM, kv, kb * BLK:(kb + 1) * BLK],
                                qt_cur[jp][jr:jr + HEAD_DIM, jh, off:],
                                start=True, stop=True)
                        # mask add ON THE PE: accumulate I.T @ mask into the
                        # score psum — avoids a DVE hop in the scores->exp->PV
                        # chain (f32r identity matmul, 1 cycle/row).
                        for ql in range(off // BLK, len(qis)):
                            qi = qis[ql]
                            if cls[qi, kb] == GENERAL:
                                csl = slice(ql * BLK, (ql + 1) * BLK)
                                for sub in range(2):
                                    nc.tensor.matmul(
                                        stp[:, sub, csl], identF[:, :],
                                        mk_all[:, idx[qi, kb], sub, :],
                                        start=False, stop=True,
                                        skip_group_check=True)
                        p = pp.tile([BLK, 2, CHUNK], DT, tag="p", name="p")
                        nc.scalar.activation(
                            p[:, :, off:], stp[:, :, off:],
                            mybir.ActivationFunctionType.Exp, scale=SCALE)
                        for ql in range(off // BLK, len(qis)):
                            if cls[qis[ql], kb] == SKIP:
                                nc.vector.memset(
                                    p[:, :, ql * BLK:(ql + 1) * BLK], 0.0)
                        return p, off

                    def emit_pv(n_kb, kb, p, off):
                        for sub in range(2):
                            nc.tensor.matmul(
                                pv_ps[0:HEAD_DIM + 1, sub, off:],
                                V_sb[kv][kb][:, :], p[:, sub, off:],
                                start=(n_kb == 0), stop=(n_kb == len(kbs) - 1))

                    # Software-pipelined: PV lags scores by `pvlag` kbs (PV only
                    # needs the SBUF p tile, so lag is free in PSUM) — hides the
                    # ACT latency + semaphore hops; filler steps (prev-chunk
                    # outproj) absorb the ACT-rate deficit.
                    lag = int(OPTS.get("pvlag", 2))
                    pop_filler()
                    pending = []
                    for n_kb, kb in enumerate(kbs):
                        p, off = emit_scores(n_kb, kb)
                        pending.append((n_kb, kb, p, off))
                        if len(pending) > lag:
                            emit_pv(*pending.pop(0))
                            pop_filler()
                    for pr in pending:
                        emit_pv(*pr)
                    # Copy PV out of PSUM immediately: frees the pv bank for the
                    # next hp-run ~4us earlier than running the whole normalize
                    # chain out of PSUM would.
                    pvs = mp.tile([HEAD_DIM + 1, 2, CHUNK], F32, tag="pvs",
                                  name="pvs")
                    nc.vector.tensor_copy(pvs[:, :, :], pv_ps[0:HEAD_DIM + 1, :, :])
                    recip = mp.tile([1, 2, CHUNK], F32, tag="recip", name="recip")
                    nc.vector.reciprocal(recip[:, :, :],
                                         pvs[HEAD_DIM:HEAD_DIM + 1, :, :])
                    bc = mp.tile([HEAD_DIM, 2, CHUNK], F32, tag="bc", name="bc")
                    nc.gpsimd.partition_broadcast(bc[:, :, :], recip[:, :, :])
                    aT = mp.tile([BLK, CHUNK], DT, tag=f"attnT{hp}",
                                 name=f"attnT{hp}")
                    for sub in range(2):
                        jr = sub * HEAD_DIM
                        mult_engine.tensor_tensor(
                            out=aT[jr:jr + HEAD_DIM, :],
                            in0=pvs[0:HEAD_DIM, sub, :], in1=bc[:, sub, :],
                            op=mybir.AluOpType.mult)
                    return aT

                def emit_outproj(qi, aTs, pools=None, defer=False):
                    ql = qi % (CHUNK // BLK)
                    state = {}

                    def st_alloc():
                        state["o"] = op.tile([BLK, 2, 2, CHUNK], BF16, tag="o",
                                             name="o_big")

                    def st_mm(eh, jt, pool, ptag):
                        if jt == 0:
                            state["acc"] = pool.tile([BLK, 2, CHUNK], F32,
                                                     tag=ptag, name="acc")
                        for ei in range(2):
                            et = eh * 2 + ei
                            nc.tensor.matmul(
                                state["acc"][:, ei, :],
                                aTs[jt][:, ql * BLK:(ql + 1) * BLK],
                                wo_all[:, jt, et * CHUNK:(et + 1) * CHUNK],
                                start=(jt == 0), stop=(jt == NJT - 1))

                    def st_copy(eh, split=False):
                        if split:
                            # halve slot turnaround: DVE + (otherwise idle) ACT
                            nc.vector.tensor_copy(state["o"][:, eh, 0, :],
                                                  state["acc"][:, 0, :])
                            nc.scalar.copy(state["o"][:, eh, 1, :],
                                           state["acc"][:, 1, :])
                        elif OPTS.get("ocopy") == "scalar":
                            nc.scalar.copy(state["o"][:, eh, :, :],
                                           state["acc"][:, :, :])
                        else:
                            nc.vector.tensor_copy(state["o"][:, eh, :, :],
                                                  state["acc"][:, :, :])

                    def st_dma():
                        nc.sync.dma_start(outD[qi * BLK:(qi + 1) * BLK, :],
                                          state["o"][:, :, :, :])

                    split = bool(pools)
                    steps = [st_alloc]
                    for eh in range(2):
                        if pools is None:
                            pool, ptag = psJ, "pj"
                        else:
                            pool, ptag = pools[eh % len(pools)]
                        for jt in range(NJT):
                            steps.append(lambda eh=eh, jt=jt, pl=pool, pt=ptag:
                                         st_mm(eh, jt, pl, pt))
                        steps.append(lambda eh=eh: st_copy(eh, split))
                        if defer:
                            # spacers: give the copy time to free the psum slot
                            # before the next acc's first matmul issues on PE
                            steps.extend([lambda: None] *
                                         int(OPTS.get("spacers", 2)))
                    steps.append(st_dma)
                    if defer:
                        filler.extend(steps)
                    else:
                        for s in steps:
                            s()

                # ---- prologue: chunk 0 projections ----
                emit_x_dma(0, split=True)
                emit_w_dmas()
                if "proj" in phases:
                    emit_projQ(0, 0)
                    emit_projQ(0, 1)
                    emit_projKV(0)

                prev_aTs = None
                prev_qis = None
                for c in range(NCH):
                    qis = list(range(c * (CHUNK // BLK), (c + 1) * (CHUNK // BLK)))
                    qt_cur = list(qt_next)
                    if c + 1 < NCH:
                        emit_x_dma(c + 1, split=True)
                    # gpsimd tensor_tensor measured ~7x slower than modeled
                    mult_eng = (nc.gpsimd if OPTS.get("mult") == "gpsimd"
                                else nc.vector)
                    block_mode = OPTS.get("ojmode", "filler") == "block"
                    projfill = bool(int(OPTS.get("projfill", 0)))
                    if projfill:
                        rrb = [(psS, "sp"), (psS, "sp")]
                    else:
                        rrb = [(psS, "sp"), (psJ, "pj")]
                    aTs = []
                    for hp in range(H_LOC // 2):
                        if "proj" in phases and c + 1 < NCH and projfill and hp == 1:
                            emit_projQ(c + 1, 0, defer=True)
                            emit_projQ(c + 1, 1, defer=True)
                        if "attn" in phases:
                            if "out" in phases and prev_aTs is not None:
                                if block_mode:
                                    if hp == 2:
                                        for i, qi in enumerate(prev_qis):
                                            emit_outproj(qi, prev_aTs,
                                                         pools=[rrb[i % 2],
                                                                rrb[(i + 1) % 2]])
                                else:
                                    emit_outproj(prev_qis[hp], prev_aTs,
                                                 defer=True)
                            aTs.append(emit_attn_hp(c, hp, qis, qt_cur, mult_eng))
                        if "proj" in phases and c + 1 < NCH and not projfill:
                            if hp == 1:
                                emit_projQ(c + 1, 0)
                            elif hp == 2:
                                emit_projQ(c + 1, 1)
                        if "proj" in phases and c + 1 < NCH and hp == 3:
                            emit_projKV(c + 1)
                    pop_filler(len(filler))   # drain before attnT ring reuse
                    prev_aTs, prev_qis = aTs, qis

                # epilogue: last chunk's outproj — attention psum is free, so
                # round-robin accs across all tags to pipeline the copies.
                if "attn" in phases and "out" in phases and prev_aTs is not None:
                    rr = [[(psS, "sp"), (psV, "pv")], [(psJ, "pj"), (psS, "sp")]]
                    for i, qi in enumerate(prev_qis):
                        emit_outproj(qi, prev_aTs, pools=rr[i % 2])

            if iters == 1:
                body()
            else:
                hints = (mybir.EngineType.PE, mybir.EngineType.DVE,
                         mybir.EngineType.Activation, mybir.EngineType.SP,
                         mybir.EngineType.Pool)
                unroll = int(OPTS.get("unroll", 1))
                if unroll > 1 and iters % unroll == 0:
                    with tc.For_i(0, iters // unroll, hint_engines=hints):
                        for _ in range(unroll):
                            body()
                else:
                    with tc.For_i(0, iters, hint_engines=hints):
                        body()
    nc.compile()
    return nc


def make_in_maps(x, wq, wk, wv, wo, ublk, compute=COMPUTE):
    npdt = ml_dtypes.bfloat16 if compute == "bf16" else np.float32
    ident = np.tile(np.eye(HEAD_DIM, dtype=np.float32), (2, 1)).astype(npdt)
    identf = np.eye(BLK, dtype=np.float32).astype(npdt)
    n_ublk = len(ublk)
    # maskP [128, n_ublk, 2, 128]: each transposed+prescaled block duplicated
    mk = np.repeat(ublk[:, None, :, :], 2, axis=1)          # [n, 2, 128, 128]
    mk = np.ascontiguousarray(mk.transpose(2, 0, 1, 3))     # [128, n, 2, 128]
    in_maps = []
    for cc in range(N_CORES):
        b, g = cc // TP, cc % TP
        xb = x[b]                                           # [S, D]
        # xP[p, c, db, j] = x[c*CHUNK+j, db*BLK+p]
        xp = xb.reshape(NCH, CHUNK, NB, BLK).transpose(3, 0, 2, 1)
        wql = wq[g * JD:(g + 1) * JD, :]                    # [JD, D]
        # wqP[p, db, jd] = wq[g*JD+jd, db*BLK+p]
        wqp = wql.reshape(JD, NB, BLK).transpose(2, 1, 0)
        wkl = wk[g * KVD:(g + 1) * KVD, :]
        wkp = wkl.reshape(KVD, NB, BLK).transpose(2, 1, 0)
        wvl = wv[g * KVD:(g + 1) * KVD, :]
        wvp = wvl.reshape(KVD, NB, BLK).transpose(2, 1, 0)
        wol = wo[:, g * JD:(g + 1) * JD]                    # [D, JD]
        # woP[p, jt, e] = wo[e, g*JD + jt*BLK + p]
        wop = wol.reshape(D, NJT, BLK).transpose(2, 1, 0)
        in_maps.append({
            "xP": np.ascontiguousarray(xp).astype(npdt),
            "wqP": np.ascontiguousarray(wqp).astype(npdt),
            "wkP": np.ascontiguousarray(wkp).astype(npdt),
            "wvP": np.ascontiguousarray(wvp).astype(npdt),
            "woP": np.ascontiguousarray(wop).astype(npdt),
            "identP": ident,
            "identFP": identf,
            "maskP": mk.astype(npdt),
        })
    return in_maps


def kernel(x, wq, wk, wv, wo, mask, start_pos):
    x = np.asarray(x, dtype=np.float32)
    wq = np.asarray(wq, dtype=np.float32)
    wk = np.asarray(wk, dtype=np.float32)
    wv = np.asarray(wv, dtype=np.float32)
    wo = np.asarray(wo, dtype=np.float32)
    mask = np.asarray(mask, dtype=np.float32)

    cls, idx, ublk = classify_mask(mask)
    nc = build_program(cls, idx, len(ublk), iters=1)
    in_maps = make_in_maps(x, wq, wk, wv, wo, ublk)
    res = run_bass_kernel_spmd(nc, in_maps, core_ids=list(range(N_CORES)),
                               trace=False)
    out = np.zeros((B, S, D), dtype=np.float32)
    for c in range(N_CORES):
        out[c // TP] += res.results[c]["out"].astype(np.float32)
    return out
